# revision 7
# baseline (speedup 1.0000x reference)
"""Causal self-attention with rotary embeddings (B=2, T=2048, D=1024, H=16,
d_k=64) on 8 Trainium2 NeuronCores.

Sharding: core c handles batch b = c//4 and 4 heads (c%4)*4..+4 — data
parallel on B, tensor parallel on heads.  Each core computes its heads'
qkv projection, RoPE, causal attention, and a partial output projection
over its 256 attention channels; the host sums the 4 partials per batch.

Layout tricks:
  * q/k channels are de-interleaved host-side (RoPE pair -> half-split
    form) and packed 2 heads per 128-partition tile; scores matmuls are
    row-tiled K=64 pairs.
  * RoPE swap (+/- sign) is a 128x128 permutation matmul on TensorE; the
    cos/sin elementwise work runs on VectorE fused with PSUM eviction.
  * softmax skips max-subtraction (scores ~ N(0,1), bounded) and folds the
    denominator into attn@v as an extra ones-column of v; the divide is a
    per-head broadcast-reciprocal multiply at eviction.
  * all matmul inputs are float32r (TF32-rate on TensorE at full fp32
    memory layout); exp outputs / v are bf16.
"""

import sys

sys.path.insert(0, "/opt/trn_rl_repo")

import numpy as np
import ml_dtypes

import concourse.bacc as bacc
import concourse.tile as tile
from concourse import mybir
from concourse.bass_utils import run_bass_kernel_spmd

F32 = mybir.dt.float32
F32R = mybir.dt.float32r
BF16 = mybir.dt.bfloat16

B, T, D = 2, 2048, 1024
NH, DK = 16, 64
THETA = 10000.0
NCORES = 8
HEADS_PER_CORE = 4

TC512 = T // 512        # 4   i-chunks of 512
TC128 = T // 128        # 16  t/j-chunks of 128
KC = D // 128           # 8   d_model contraction chunks


def build_program(debug=False):
    nc = bacc.Bacc("TRN2", target_bir_lowering=False, debug=False)

    XT = nc.dram_tensor("XT", [D + 1, T], F32R, kind="ExternalInput").ap()
    WQK = nc.dram_tensor("WQK", [D + 1, 512], F32R, kind="ExternalInput").ap()
    WV = nc.dram_tensor("WV", [D + 1, 256], F32R, kind="ExternalInput").ap()
    WOUT = nc.dram_tensor("WOUT", [256, D], F32R, kind="ExternalInput").ap()
    PSW = nc.dram_tensor("PSW", [128, 128], F32R, kind="ExternalInput").ap()
    CQ = nc.dram_tensor("CQ", [128, T], F32, kind="ExternalInput").ap()
    SQ = nc.dram_tensor("SQ", [128, T], F32, kind="ExternalInput").ap()
    TRI = nc.dram_tensor("TRI", [128, 128], BF16, kind="ExternalInput").ap()
    ONES64 = nc.dram_tensor("ONES64", [1, 64], F32R, kind="ExternalInput").ap()
    OUT = nc.dram_tensor("OUT", [T, D], F32, kind="ExternalOutput").ap()
    if debug:
        DBG_QKT = nc.dram_tensor("DBG_QKT", [128, 4 * T], F32, kind="ExternalOutput").ap()
        DBG_V = nc.dram_tensor("DBG_V", [128, TC128 * 260], BF16, kind="ExternalOutput").ap()
        DBG_ATT = nc.dram_tensor("DBG_ATT", [128, 2 * T], F32, kind="ExternalOutput").ap()

    MUL = mybir.AluOpType.mult
    EXP = mybir.ActivationFunctionType.Exp

    with tile.TileContext(nc) as tc:
        with (
            tc.tile_pool(name="persist", bufs=1) as persist,
            tc.tile_pool(name="p2sb", bufs=4) as p2sb,
        ):
            # ---- persistent tiles --------------------------------------
            qkT = persist.tile([128, 4 * T], F32R, tag="qkT")       # Qp0 Kp0 Qp1 Kp1
            v_sb = persist.tile([128, TC128 * 260], BF16, tag="v_sb")  # [jc, head, 64+1]
            attnT = persist.tile([128, 2 * T], F32R, tag="attnT")   # c-chunks x t
            wout_sb = persist.tile([128, 2 * D], F32R, tag="wout_sb")
            tri_sb = persist.tile([128, 128], BF16, tag="tri_sb")
            ones64_sb = persist.tile([1, 64], F32R, tag="ones64_sb")

            nc.sync.dma_start(tri_sb[:], TRI[:])
            nc.sync.dma_start(ones64_sb[:], ONES64[:])
            for cc in range(2):
                nc.sync.dma_start(wout_sb[:, cc * D:(cc + 1) * D], WOUT[cc * 128:(cc + 1) * 128, :])

            # ones columns of v_aug: one strided memset
            v4 = v_sb[:].rearrange("p (jc h e) -> p jc h e", jc=TC128, h=4)
            nc.vector.memset(v4[:, :, :, 64:65], 1.0)

            # ================= phase 1: qkv projection + rope ===========
            with (
                tc.tile_pool(name="p1w", bufs=1) as p1w,
                tc.tile_pool(name="p1t", bufs=3) as p1t,
                tc.tile_pool(name="p1ps", bufs=2, space="PSUM") as p1ps,
                tc.tile_pool(name="p1ps2", bufs=2, space="PSUM") as p1ps2,
            ):
                x_sb = p1w.tile([128, KC * T], F32R, tag="x_sb")
                xlast = p1w.tile([1, T], F32R, tag="xlast")
                wqk_sb = p1w.tile([128, KC * 512], F32R, tag="wqk_sb")
                wqk_last = p1w.tile([1, 512], F32R, tag="wqk_last")
                wv_sb = p1w.tile([128, KC * 256], F32R, tag="wv_sb")
                wv_last = p1w.tile([1, 256], F32R, tag="wv_last")
                psw_sb = p1w.tile([128, 128], F32R, tag="psw_sb")
                cq_sb = p1w.tile([128, T], F32, tag="cq_sb")
                sq_sb = p1w.tile([128, T], F32, tag="sq_sb")

                for k in range(KC):
                    nc.sync.dma_start(x_sb[:, k * T:(k + 1) * T], XT[k * 128:(k + 1) * 128, :])
                    nc.sync.dma_start(wqk_sb[:, k * 512:(k + 1) * 512], WQK[k * 128:(k + 1) * 128, :])
                    nc.sync.dma_start(wv_sb[:, k * 256:(k + 1) * 256], WV[k * 128:(k + 1) * 128, :])
                nc.sync.dma_start(xlast[:], XT[D:D + 1, :])
                nc.sync.dma_start(wqk_last[:], WQK[D:D + 1, :])
                nc.sync.dma_start(wv_last[:], WV[D:D + 1, :])
                nc.sync.dma_start(psw_sb[:], PSW[:])
                nc.sync.dma_start(cq_sb[:], CQ[:])
                nc.sync.dma_start(sq_sb[:], SQ[:])

                # q/k projection: out[m-chunk of 128 channels, n-chunk of 512 t]
                for m in range(4):
                    is_q = (m % 2 == 0)
                    for n in range(TC512):
                        nsl = slice(n * 512, (n + 1) * 512)
                        ps = p1ps.tile([128, 512], F32, tag="ps_qk")
                        for k in range(KC):
                            nc.tensor.matmul(
                                ps[:],
                                wqk_sb[:, k * 512 + m * 128:k * 512 + (m + 1) * 128],
                                x_sb[:, k * T + n * 512:k * T + (n + 1) * 512],
                                start=(k == 0), stop=False,
                            )
                        nc.tensor.matmul(
                            ps[:], wqk_last[:, m * 128:(m + 1) * 128], xlast[:, nsl],
                            start=False, stop=True,
                        )
                        # rope: qkT = C*ps + P @ (S*ps)   (q tables pre-scaled 1/8)
                        tmp_s = p1t.tile([128, 512], F32R, tag="tmp_s")
                        tmp_c = p1t.tile([128, 512], F32, tag="tmp_c")
                        if is_q:
                            nc.vector.tensor_mul(tmp_s[:], ps[:], sq_sb[:, nsl])
                            nc.vector.tensor_mul(tmp_c[:], ps[:], cq_sb[:, nsl])
                        else:
                            nc.vector.scalar_tensor_tensor(tmp_s[:], ps[:], 8.0, sq_sb[:, nsl], MUL, MUL)
                            nc.vector.scalar_tensor_tensor(tmp_c[:], ps[:], 8.0, cq_sb[:, nsl], MUL, MUL)
                        sw = p1ps2.tile([128, 512], F32, tag="sw")
                        nc.tensor.matmul(sw[:], psw_sb[:], tmp_s[:], start=True, stop=True)
                        nc.vector.tensor_add(qkT[:, m * T + n * 512:m * T + (n + 1) * 512], sw[:], tmp_c[:])

                # v projection: out[t-chunk of 128, 4*64 channels] -> bf16 v_sb
                for tcc in range(TC128):
                    tsl = slice(tcc * 128, (tcc + 1) * 128)
                    psv = p1ps.tile([128, 256], F32, tag="ps_qk", name=f"psv_{tcc}")
                    for k in range(KC):
                        nc.tensor.matmul(
                            psv[:],
                            x_sb[:, k * T + tcc * 128:k * T + (tcc + 1) * 128],
                            wv_sb[:, k * 256:(k + 1) * 256],
                            start=(k == 0), stop=False,
                        )
                    nc.tensor.matmul(psv[:], xlast[:, tsl], wv_last[:], start=False, stop=True)
                    for h in range(4):
                        nc.vector.tensor_copy(
                            v_sb[:, tcc * 260 + h * 65:tcc * 260 + h * 65 + 64],
                            psv[:, h * 64:(h + 1) * 64],
                        )

            # ================= phase 2: attention =======================
            with (
                tc.tile_pool(name="p2e", bufs=6) as p2e,
                tc.tile_pool(name="p2bc", bufs=2) as p2bc,
                tc.tile_pool(name="sps", bufs=3, space="PSUM") as sps,
                tc.tile_pool(name="avps", bufs=2, space="PSUM") as avps,
                tc.tile_pool(name="bcps", bufs=1, space="PSUM") as bcps,
            ):
                for p in range(2):
                    qof = (2 * p) * T
                    kof = (2 * p + 1) * T
                    for ic in range(TC512):
                        isl = slice(ic * 512, (ic + 1) * 512)
                        njc = 4 * ic + 4
                        av = [avps.tile([65, 512], F32, tag="av", name=f"av_{p}_{ic}_{i}") for i in range(2)]
                        for jc in range(njc):
                            jsl = slice(jc * 128, (jc + 1) * 128)
                            rel = jc - 4 * ic
                            e_t = [p2e.tile([128, 512], BF16, tag="e_t", name=f"e_{p}_{ic}_{jc}_{i}") for i in range(2)]
                            for hh in range(2):
                                pof = hh * 64
                                s_ps = sps.tile([128, 512], F32, tag="s_ps")
                                nc.tensor.matmul(
                                    s_ps[:],
                                    qkT[pof:pof + 64, kof + jc * 128:kof + (jc + 1) * 128],
                                    qkT[pof:pof + 64, qof + ic * 512:qof + (ic + 1) * 512],
                                    start=True, stop=True,
                                )
                                ls = 0 if rel < 0 else rel * 128
                                if ls > 0:
                                    nc.gpsimd.memset(e_t[hh][:, 0:ls], 0.0)
                                nc.scalar.activation(e_t[hh][:, ls:512], s_ps[:, ls:512], EXP)
                                if rel >= 0:
                                    tri_slice = slice(rel * 128, (rel + 1) * 128)
                                    nc.vector.tensor_mul(e_t[hh][:, tri_slice], e_t[hh][:, tri_slice], tri_sb[:])
                                nc.tensor.matmul(
                                    av[hh][:],
                                    v_sb[:, jc * 260 + (2 * p + hh) * 65:jc * 260 + (2 * p + hh) * 65 + 65],
                                    e_t[hh][:],
                                    start=(jc == 0), stop=(jc == njc - 1),
                                    skip_group_check=True,
                                )
                        for hh in range(2):
                            head = 2 * p + hh
                            rec = p2bc.tile([1, 512], F32R, tag="rec")
                            with nc.allow_low_precision(reason="denominator reciprocal in f32r for matmul broadcast"):
                                nc.vector.reciprocal(rec[:], av[hh][64:65, :])
                            bc_ps = bcps.tile([64, 512], F32, tag="bc_ps")
                            nc.tensor.matmul(bc_ps[:], ones64_sb[:], rec[:], start=True, stop=True)
                            bc_sb = p2bc.tile([64, 512], F32, tag="bc_sb")
                            nc.vector.tensor_copy(bc_sb[:], bc_ps[:])
                            cof = (head // 2) * T
                            pof = (head % 2) * 64
                            nc.vector.tensor_mul(
                                attnT[pof:pof + 64, cof + ic * 512:cof + (ic + 1) * 512],
                                av[hh][0:64, :], bc_sb[:],
                            )

            # ================= phase 3: output projection ===============
            with tc.tile_pool(name="p3ps", bufs=4, space="PSUM") as p3ps:
              for tcc in range(TC128):
                tsl = slice(tcc * 128, (tcc + 1) * 128)
                for oc in range(2):
                    osl = slice(oc * 512, (oc + 1) * 512)
                    po = p3ps.tile([128, 512], F32, tag="po", name=f"po_{tcc}_{oc}")
                    for cc in range(2):
                        nc.tensor.matmul(
                            po[:],
                            attnT[:, cc * T + tcc * 128:cc * T + (tcc + 1) * 128],
                            wout_sb[:, cc * D + oc * 512:cc * D + (oc + 1) * 512],
                            start=(cc == 0), stop=(cc == 1),
                        )
                    po_sb = p2sb.tile([128, 512], F32, tag="po_sb", name=f"po_sb_{tcc}_{oc}")
                    if oc == 0:
                        nc.vector.tensor_copy(po_sb[:], po[:])
                    else:
                        nc.scalar.copy(po_sb[:], po[:])
                    nc.sync.dma_start(OUT[tsl, osl], po_sb[:])

            if debug:
                nc.sync.dma_start(DBG_QKT[:], qkT[:].bitcast(F32))
                nc.sync.dma_start(DBG_V[:], v_sb[:])
                nc.sync.dma_start(DBG_ATT[:], attnT[:].bitcast(F32))

    nc.compile()
    return nc


_DEINT = list(range(0, DK, 2)) + list(range(1, DK, 2))


def _rope_tables():
    j = np.arange(DK // 2, dtype=np.float64)
    inv_freq = THETA ** (-2.0 * j / DK)
    t = np.arange(T, dtype=np.float64)
    ang = t[None, :] * inv_freq[:, None]          # [32, T]
    ang = np.tile(ang, (4, 1))                    # [128, T]
    return np.cos(ang).astype(np.float32), np.sin(ang).astype(np.float32)


def _psw():
    M = np.zeros((128, 128), dtype=np.float32)
    for p in range(128):
        pm = p % 64
        if pm < 32:
            M[p, p + 32] = -1.0
        else:
            M[p, p - 32] = 1.0
    return np.ascontiguousarray(M.T)


def shard_inputs(x, Wqkv, bqkv, Wout, bout):
    x = np.asarray(x, dtype=np.float32)
    Wqkv = np.asarray(Wqkv, dtype=np.float32)
    bqkv = np.asarray(bqkv, dtype=np.float32)
    Wout = np.asarray(Wout, dtype=np.float32)

    cos_t, sin_t = _rope_tables()
    cq = np.ascontiguousarray(cos_t / 8.0)
    sq = np.ascontiguousarray(sin_t / 8.0)
    psw = _psw()
    tri = np.triu(np.ones((128, 128), dtype=np.float32)).astype(ml_dtypes.bfloat16)
    ones64 = np.ones((1, 64), dtype=np.float32)

    Wfull = np.concatenate([Wqkv, bqkv[:, None]], axis=1)  # [3072, 1025]

    xt = {}
    for b in range(B):
        xt[b] = np.ascontiguousarray(
            np.concatenate([x[b].T, np.ones((1, T), np.float32)], axis=0)
        )

    in_maps = []
    for c in range(NCORES):
        b = c // 4
        heads = [4 * (c % 4) + i for i in range(HEADS_PER_CORE)]
        # chunk order: [Qp0 | Kp0 | Qp1 | Kp1], each 128 rows (2 heads x 64)
        qk_rows = []
        for p in range(2):
            qrows, krows = [], []
            for h in (2 * p, 2 * p + 1):
                H = heads[h]
                qrows += [H * 192 + j for j in _DEINT]
                krows += [H * 192 + 64 + j for j in _DEINT]
            qk_rows += qrows + krows
        v_rows = []
        for h in range(4):
            H = heads[h]
            v_rows += [H * 192 + 128 + j for j in range(DK)]
        vch_out = []
        for h in range(4):
            H = heads[h]
            vch_out += [H * 64 + j for j in range(DK)]

        in_maps.append({
            "XT": xt[b],
            "WQK": np.ascontiguousarray(Wfull[qk_rows].T),
            "WV": np.ascontiguousarray(Wfull[v_rows].T),
            "WOUT": np.ascontiguousarray(Wout[:, vch_out].T),
            "PSW": psw,
            "CQ": cq,
            "SQ": sq,
            "TRI": tri,
            "ONES64": ones64,
        })
    return in_maps


_CACHED = {}


def _get_program(debug=False):
    key = bool(debug)
    if key not in _CACHED:
        _CACHED[key] = build_program(debug=debug)
    return _CACHED[key]


def run_cores(inputs, debug=False, trace=False, tmpdir=None):
    nc = _get_program(debug=debug)
    in_maps = shard_inputs(**inputs)
    res = run_bass_kernel_spmd(
        nc, in_maps, core_ids=list(range(NCORES)), trace=trace, tmpdir=tmpdir,
    )
    return res


def combine(results, bout):
    bout = np.asarray(bout, dtype=np.float32)
    out = np.empty((B, T, D), dtype=np.float32)
    for b in range(B):
        acc = results[4 * b]["OUT"].astype(np.float32).copy()
        for c in range(4 * b + 1, 4 * b + 4):
            acc += results[c]["OUT"]
        out[b] = acc + bout[None, :]
    return out


def kernel(x, Wqkv, bqkv, Wout, bout):
    res = run_cores(dict(x=x, Wqkv=Wqkv, bqkv=bqkv, Wout=Wout, bout=bout))
    return combine(res.results, bout)


# revision 9
# speedup vs baseline: 1.1544x; 1.1544x over previous
"""Causal self-attention with rotary embeddings (B=2, T=2048, D=1024, H=16,
d_k=64) on 8 Trainium2 NeuronCores.

Sharding: core c handles batch b = c//4 and 4 heads (c%4)*4..+4 — data
parallel on B, tensor parallel on heads.  Each core computes its heads'
qkv projection, RoPE, causal attention, and a partial output projection
over its 256 attention channels; the host sums the 4 partials per batch.

Layout tricks:
  * q/k channels are de-interleaved host-side (RoPE pair -> half-split
    form) and packed 2 heads per 128-partition tile; scores matmuls are
    row-tiled K=64 pairs.
  * RoPE swap (+/- sign) is a 128x128 permutation matmul on TensorE; the
    cos/sin elementwise work runs on VectorE fused with PSUM eviction.
  * softmax skips max-subtraction (scores ~ N(0,1), bounded) and folds the
    denominator into attn@v as an extra ones-column of v; the divide is a
    per-head broadcast-reciprocal multiply at eviction.
  * all matmul inputs are float32r (TF32-rate on TensorE at full fp32
    memory layout); exp outputs / v are bf16.
"""

import sys

sys.path.insert(0, "/opt/trn_rl_repo")

import numpy as np
import ml_dtypes

import concourse.bacc as bacc
import concourse.tile as tile
from concourse import mybir
from concourse.bass_utils import run_bass_kernel_spmd

F32 = mybir.dt.float32
F32R = mybir.dt.float32r
BF16 = mybir.dt.bfloat16

B, T, D = 2, 2048, 1024
NH, DK = 16, 64
THETA = 10000.0
NCORES = 8
HEADS_PER_CORE = 4

TC512 = T // 512        # 4   i-chunks of 512
TC128 = T // 128        # 16  t/j-chunks of 128
KC = D // 128           # 8   d_model contraction chunks


def build_program(debug=False):
    nc = bacc.Bacc("TRN2", target_bir_lowering=False, debug=False)

    XT = nc.dram_tensor("XT", [D + 1, T], F32R, kind="ExternalInput").ap()
    WQK = nc.dram_tensor("WQK", [D + 1, 512], F32R, kind="ExternalInput").ap()
    WV = nc.dram_tensor("WV", [D + 1, 256], F32R, kind="ExternalInput").ap()
    WOUT = nc.dram_tensor("WOUT", [256, D], F32R, kind="ExternalInput").ap()
    PSW = nc.dram_tensor("PSW", [128, 128], F32R, kind="ExternalInput").ap()
    CQ = nc.dram_tensor("CQ", [128, T], F32, kind="ExternalInput").ap()
    SQ = nc.dram_tensor("SQ", [128, T], F32, kind="ExternalInput").ap()
    TRI = nc.dram_tensor("TRI", [128, 128], BF16, kind="ExternalInput").ap()
    ONES64 = nc.dram_tensor("ONES64", [1, 64], F32R, kind="ExternalInput").ap()
    OUT = nc.dram_tensor("OUT", [T, D], F32, kind="ExternalOutput").ap()
    if debug:
        DBG_QKT = nc.dram_tensor("DBG_QKT", [128, 4 * T], F32, kind="ExternalOutput").ap()
        DBG_V = nc.dram_tensor("DBG_V", [128, TC128 * 260], BF16, kind="ExternalOutput").ap()
        DBG_ATT = nc.dram_tensor("DBG_ATT", [128, 2 * T], F32, kind="ExternalOutput").ap()

    MUL = mybir.AluOpType.mult
    EXP = mybir.ActivationFunctionType.Exp

    with tile.TileContext(nc) as tc:
        with (
            tc.tile_pool(name="persist", bufs=1) as persist,
            tc.tile_pool(name="p2sb", bufs=4) as p2sb,
        ):
            # ---- persistent tiles --------------------------------------
            qkT = persist.tile([128, 4 * T], F32R, tag="qkT")       # Qp0 Kp0 Qp1 Kp1
            v_sb = persist.tile([128, TC128 * 260], BF16, tag="v_sb")  # [jc, head, 64+1]
            attnT = persist.tile([128, 2 * T], F32R, tag="attnT")   # c-chunks x t
            wout_sb = persist.tile([128, 2 * D], F32R, tag="wout_sb")
            tri_sb = persist.tile([128, 128], BF16, tag="tri_sb")
            ones64_sb = persist.tile([1, 64], F32R, tag="ones64_sb")

            nc.sync.dma_start(tri_sb[:], TRI[:])
            nc.sync.dma_start(ones64_sb[:], ONES64[:])
            for cc in range(2):
                nc.sync.dma_start(wout_sb[:, cc * D:(cc + 1) * D], WOUT[cc * 128:(cc + 1) * 128, :])

            # ones columns of v_aug: one strided memset
            v4 = v_sb[:].rearrange("p (jc h e) -> p jc h e", jc=TC128, h=4)
            nc.vector.memset(v4[:, :, :, 64:65], 1.0)

            # ================= phase 1: qkv projection + rope ===========
            with (
                tc.tile_pool(name="p1w", bufs=1) as p1w,
                tc.tile_pool(name="p1t", bufs=4) as p1t,
                tc.tile_pool(name="p1ps", bufs=3, space="PSUM") as p1ps,
                tc.tile_pool(name="p1ps2", bufs=2, space="PSUM") as p1ps2,
            ):
                x_sb = p1w.tile([128, KC * T], F32R, tag="x_sb")
                xlast = p1w.tile([1, T], F32R, tag="xlast")
                wqk_sb = p1w.tile([128, KC * 512], F32R, tag="wqk_sb")
                wqk_last = p1w.tile([1, 512], F32R, tag="wqk_last")
                wv_sb = p1w.tile([128, KC * 256], F32R, tag="wv_sb")
                wv_last = p1w.tile([1, 256], F32R, tag="wv_last")
                psw_sb = p1w.tile([128, 128], F32R, tag="psw_sb")
                cq_sb = p1w.tile([128, T], F32, tag="cq_sb")
                sq_sb = p1w.tile([128, T], F32, tag="sq_sb")

                for k in range(KC):
                    nc.sync.dma_start(x_sb[:, k * T:(k + 1) * T], XT[k * 128:(k + 1) * 128, :])
                    nc.sync.dma_start(wqk_sb[:, k * 512:(k + 1) * 512], WQK[k * 128:(k + 1) * 128, :])
                    nc.sync.dma_start(wv_sb[:, k * 256:(k + 1) * 256], WV[k * 128:(k + 1) * 128, :])
                nc.sync.dma_start(xlast[:], XT[D:D + 1, :])
                nc.sync.dma_start(wqk_last[:], WQK[D:D + 1, :])
                nc.sync.dma_start(wv_last[:], WV[D:D + 1, :])
                nc.sync.dma_start(psw_sb[:], PSW[:])
                nc.sync.dma_start(cq_sb[:], CQ[:])
                nc.sync.dma_start(sq_sb[:], SQ[:])

                # q/k projection: out[m-chunk of 128 channels, n-chunk of 512 t]
                for m in range(4):
                    is_q = (m % 2 == 0)
                    for n in range(TC512):
                        nsl = slice(n * 512, (n + 1) * 512)
                        ps = p1ps.tile([128, 512], F32, tag="ps_qk")
                        for k in range(KC):
                            nc.tensor.matmul(
                                ps[:],
                                wqk_sb[:, k * 512 + m * 128:k * 512 + (m + 1) * 128],
                                x_sb[:, k * T + n * 512:k * T + (n + 1) * 512],
                                start=(k == 0), stop=False,
                            )
                        nc.tensor.matmul(
                            ps[:], wqk_last[:, m * 128:(m + 1) * 128], xlast[:, nsl],
                            start=False, stop=True,
                        )
                        # rope: qkT = C*ps + P @ (S*ps)   (q tables pre-scaled 1/8)
                        tmp_s = p1t.tile([128, 512], F32R, tag="tmp_s")
                        tmp_c = p1t.tile([128, 512], F32, tag="tmp_c")
                        if is_q:
                            nc.vector.tensor_mul(tmp_s[:], ps[:], sq_sb[:, nsl])
                            nc.vector.tensor_mul(tmp_c[:], ps[:], cq_sb[:, nsl])
                        else:
                            nc.vector.scalar_tensor_tensor(tmp_s[:], ps[:], 8.0, sq_sb[:, nsl], MUL, MUL)
                            nc.vector.scalar_tensor_tensor(tmp_c[:], ps[:], 8.0, cq_sb[:, nsl], MUL, MUL)
                        sw = p1ps2.tile([128, 512], F32, tag="sw")
                        nc.tensor.matmul(sw[:], psw_sb[:], tmp_s[:], start=True, stop=True)
                        nc.vector.tensor_add(qkT[:, m * T + n * 512:m * T + (n + 1) * 512], sw[:], tmp_c[:])

                # v projection: out[t-chunk of 128, 4*64 channels] -> bf16 v_sb
                for tcc in range(TC128):
                    tsl = slice(tcc * 128, (tcc + 1) * 128)
                    psv = p1ps.tile([128, 256], F32, tag="ps_qk", name=f"psv_{tcc}")
                    for k in range(KC):
                        nc.tensor.matmul(
                            psv[:],
                            x_sb[:, k * T + tcc * 128:k * T + (tcc + 1) * 128],
                            wv_sb[:, k * 256:(k + 1) * 256],
                            start=(k == 0), stop=False,
                        )
                    nc.tensor.matmul(psv[:], xlast[:, tsl], wv_last[:], start=False, stop=True)
                    vdst = v_sb[:, tcc * 260:(tcc + 1) * 260].rearrange(
                        "p (h e) -> p h e", h=4)[:, :, 0:64]
                    vsrc = psv[:].rearrange("p (h e) -> p h e", e=64)
                    nc.vector.tensor_copy(vdst, vsrc)

            # ================= phase 2: attention =======================
            with (
                tc.tile_pool(name="p2e", bufs=8) as p2e,
                tc.tile_pool(name="p2bc", bufs=2) as p2bc,
                tc.tile_pool(name="sps", bufs=3, space="PSUM") as sps,
                tc.tile_pool(name="avps", bufs=4, space="PSUM") as avps,
            ):
                for p in range(2):
                    qof = (2 * p) * T
                    kof = (2 * p + 1) * T
                    for ic in range(TC512):
                        isl = slice(ic * 512, (ic + 1) * 512)
                        njc = 4 * ic + 4
                        av = [avps.tile([65, 512], F32, tag="av", name=f"av_{p}_{ic}_{i}") for i in range(2)]
                        for jc in range(njc):
                            jsl = slice(jc * 128, (jc + 1) * 128)
                            rel = jc - 4 * ic
                            e_t = [p2e.tile([128, 512], BF16, tag="e_t", name=f"e_{p}_{ic}_{jc}_{i}") for i in range(2)]
                            ls = 0 if rel < 0 else rel * 128
                            for hh in range(2):
                                pof = hh * 64
                                s_ps = sps.tile([128, 512], F32, tag="s_ps", name=f"s_{p}_{ic}_{jc}_{hh}")
                                nc.tensor.matmul(
                                    s_ps[:],
                                    qkT[pof:pof + 64, kof + jc * 128:kof + (jc + 1) * 128],
                                    qkT[pof:pof + 64, qof + ic * 512:qof + (ic + 1) * 512],
                                    start=True, stop=True,
                                )
                                if ls > 0:
                                    nc.gpsimd.memset(e_t[hh][:, 0:ls], 0.0)
                                nc.scalar.activation(e_t[hh][:, ls:512], s_ps[:, ls:512], EXP)
                                if rel >= 0:
                                    tri_slice = slice(rel * 128, (rel + 1) * 128)
                                    nc.vector.tensor_mul(e_t[hh][:, tri_slice], e_t[hh][:, tri_slice], tri_sb[:])
                            for hh in range(2):
                                nc.tensor.matmul(
                                    av[hh][:],
                                    v_sb[:, jc * 260 + (2 * p + hh) * 65:jc * 260 + (2 * p + hh) * 65 + 65],
                                    e_t[hh][:],
                                    start=(jc == 0), stop=(jc == njc - 1),
                                    skip_group_check=True,
                                )
                        for hh in range(2):
                            head = 2 * p + hh
                            den = p2bc.tile([1, 512], F32, tag="den", name=f"den_{p}_{ic}_{hh}")
                            nc.vector.tensor_copy(den[:], av[hh][64:65, :])
                            rec = p2bc.tile([1, 512], F32, tag="rec", name=f"rec_{p}_{ic}_{hh}")
                            rsc = p2bc.tile([1, 512], F32, tag="rsc", name=f"rsc_{p}_{ic}_{hh}")
                            nc.vector.reciprocal_approx_accurate(rec[:], den[:], rsc[:])
                            bc_sb = p2bc.tile([64, 512], F32, tag="bc_sb", name=f"bc_{p}_{ic}_{hh}")
                            nc.gpsimd.partition_broadcast(bc_sb[:], rec[:], channels=64)
                            cof = (head // 2) * T
                            pof = (head % 2) * 64
                            nc.vector.tensor_mul(
                                attnT[pof:pof + 64, cof + ic * 512:cof + (ic + 1) * 512],
                                av[hh][0:64, :], bc_sb[:],
                            )

            # ================= phase 3: output projection ===============
            with tc.tile_pool(name="p3ps", bufs=4, space="PSUM") as p3ps:
              for tcc in range(TC128):
                tsl = slice(tcc * 128, (tcc + 1) * 128)
                pos = [p3ps.tile([128, 512], F32, tag="po", name=f"po_{tcc}_{oc}") for oc in range(2)]
                for cc in range(2):
                    for oc in range(2):
                        nc.tensor.matmul(
                            pos[oc][:],
                            attnT[:, cc * T + tcc * 128:cc * T + (tcc + 1) * 128],
                            wout_sb[:, cc * D + oc * 512:cc * D + (oc + 1) * 512],
                            start=(cc == 0), stop=(cc == 1),
                        )
                for oc in range(2):
                    osl = slice(oc * 512, (oc + 1) * 512)
                    po_sb = p2sb.tile([128, 512], F32, tag="po_sb", name=f"po_sb_{tcc}_{oc}")
                    if oc == 0:
                        nc.vector.tensor_copy(po_sb[:], pos[oc][:])
                    else:
                        nc.scalar.copy(po_sb[:], pos[oc][:])
                    nc.sync.dma_start(OUT[tsl, osl], po_sb[:])

            if debug:
                nc.sync.dma_start(DBG_QKT[:], qkT[:].bitcast(F32))
                nc.sync.dma_start(DBG_V[:], v_sb[:])
                nc.sync.dma_start(DBG_ATT[:], attnT[:].bitcast(F32))

    nc.compile()
    return nc


_DEINT = list(range(0, DK, 2)) + list(range(1, DK, 2))


def _rope_tables():
    j = np.arange(DK // 2, dtype=np.float64)
    inv_freq = THETA ** (-2.0 * j / DK)
    t = np.arange(T, dtype=np.float64)
    ang = t[None, :] * inv_freq[:, None]          # [32, T]
    ang = np.tile(ang, (4, 1))                    # [128, T]
    return np.cos(ang).astype(np.float32), np.sin(ang).astype(np.float32)


def _psw():
    M = np.zeros((128, 128), dtype=np.float32)
    for p in range(128):
        pm = p % 64
        if pm < 32:
            M[p, p + 32] = -1.0
        else:
            M[p, p - 32] = 1.0
    return np.ascontiguousarray(M.T)


def shard_inputs(x, Wqkv, bqkv, Wout, bout):
    x = np.asarray(x, dtype=np.float32)
    Wqkv = np.asarray(Wqkv, dtype=np.float32)
    bqkv = np.asarray(bqkv, dtype=np.float32)
    Wout = np.asarray(Wout, dtype=np.float32)

    cos_t, sin_t = _rope_tables()
    cq = np.ascontiguousarray(cos_t / 8.0)
    sq = np.ascontiguousarray(sin_t / 8.0)
    psw = _psw()
    tri = np.triu(np.ones((128, 128), dtype=np.float32)).astype(ml_dtypes.bfloat16)
    ones64 = np.ones((1, 64), dtype=np.float32)

    Wfull = np.concatenate([Wqkv, bqkv[:, None]], axis=1)  # [3072, 1025]

    xt = {}
    for b in range(B):
        xt[b] = np.ascontiguousarray(
            np.concatenate([x[b].T, np.ones((1, T), np.float32)], axis=0)
        )

    in_maps = []
    for c in range(NCORES):
        b = c // 4
        heads = [4 * (c % 4) + i for i in range(HEADS_PER_CORE)]
        # chunk order: [Qp0 | Kp0 | Qp1 | Kp1], each 128 rows (2 heads x 64)
        qk_rows = []
        for p in range(2):
            qrows, krows = [], []
            for h in (2 * p, 2 * p + 1):
                H = heads[h]
                qrows += [H * 192 + j for j in _DEINT]
                krows += [H * 192 + 64 + j for j in _DEINT]
            qk_rows += qrows + krows
        v_rows = []
        for h in range(4):
            H = heads[h]
            v_rows += [H * 192 + 128 + j for j in range(DK)]
        vch_out = []
        for h in range(4):
            H = heads[h]
            vch_out += [H * 64 + j for j in range(DK)]

        in_maps.append({
            "XT": xt[b],
            "WQK": np.ascontiguousarray(Wfull[qk_rows].T),
            "WV": np.ascontiguousarray(Wfull[v_rows].T),
            "WOUT": np.ascontiguousarray(Wout[:, vch_out].T),
            "PSW": psw,
            "CQ": cq,
            "SQ": sq,
            "TRI": tri,
            "ONES64": ones64,
        })
    return in_maps


_CACHED = {}


def _get_program(debug=False):
    key = bool(debug)
    if key not in _CACHED:
        _CACHED[key] = build_program(debug=debug)
    return _CACHED[key]


def run_cores(inputs, debug=False, trace=False, tmpdir=None):
    nc = _get_program(debug=debug)
    in_maps = shard_inputs(**inputs)
    res = run_bass_kernel_spmd(
        nc, in_maps, core_ids=list(range(NCORES)), trace=trace, tmpdir=tmpdir,
    )
    return res


def combine(results, bout):
    bout = np.asarray(bout, dtype=np.float32)
    out = np.empty((B, T, D), dtype=np.float32)
    for b in range(B):
        acc = results[4 * b]["OUT"].astype(np.float32).copy()
        for c in range(4 * b + 1, 4 * b + 4):
            acc += results[c]["OUT"]
        out[b] = acc + bout[None, :]
    return out


def kernel(x, Wqkv, bqkv, Wout, bout):
    res = run_cores(dict(x=x, Wqkv=Wqkv, bqkv=bqkv, Wout=Wout, bout=bout))
    return combine(res.results, bout)


# revision 13
# speedup vs baseline: 1.2564x; 1.0884x over previous
"""Causal self-attention with rotary embeddings (B=2, T=2048, D=1024, H=16,
d_k=64) on 8 Trainium2 NeuronCores.

Sharding: core c handles batch b = c//4 and 4 heads (c%4)*4..+4 — data
parallel on B, tensor parallel on heads.  Each core computes its heads'
qkv projection, RoPE, causal attention, and a partial output projection
over its 256 attention channels; the host sums the 4 partials per batch.

Layout tricks:
  * q/k channels are de-interleaved host-side (RoPE pair -> half-split
    form) and packed 2 heads per 128-partition tile; scores matmuls are
    row-tiled K=64 pairs.
  * RoPE swap (+/- sign) is a 128x128 permutation matmul on TensorE; the
    cos/sin elementwise work runs on VectorE fused with PSUM eviction.
  * softmax skips max-subtraction (scores ~ N(0,1), bounded) and folds the
    denominator into attn@v as an extra ones-column of v; the divide is a
    per-head broadcast-reciprocal multiply at eviction.
  * all matmul inputs are float32r (TF32-rate on TensorE at full fp32
    memory layout); exp outputs / v are bf16.
"""

import sys

sys.path.insert(0, "/opt/trn_rl_repo")

import numpy as np
import ml_dtypes

import concourse.bacc as bacc
import concourse.tile as tile
from concourse import mybir
from concourse.bass_utils import run_bass_kernel_spmd

F32 = mybir.dt.float32
F32R = mybir.dt.float32r
BF16 = mybir.dt.bfloat16

B, T, D = 2, 2048, 1024
NH, DK = 16, 64
THETA = 10000.0
NCORES = 8
HEADS_PER_CORE = 4

TC512 = T // 512        # 4   i-chunks of 512
TC128 = T // 128        # 16  t/j-chunks of 128
KC = D // 128           # 8   d_model contraction chunks


def build_program(debug=False):
    nc = bacc.Bacc("TRN2", target_bir_lowering=False, debug=False)

    XT = nc.dram_tensor("XT", [D + 1, T], F32R, kind="ExternalInput").ap()
    WQK = nc.dram_tensor("WQK", [D + 1, 512], F32R, kind="ExternalInput").ap()
    WV = nc.dram_tensor("WV", [D + 1, 256], F32R, kind="ExternalInput").ap()
    WOUT = nc.dram_tensor("WOUT", [256, D], F32R, kind="ExternalInput").ap()
    PSW = nc.dram_tensor("PSW", [128, 128], F32R, kind="ExternalInput").ap()
    CQ = nc.dram_tensor("CQ", [128, T], F32, kind="ExternalInput").ap()
    SQ = nc.dram_tensor("SQ", [128, T], F32, kind="ExternalInput").ap()
    TRI = nc.dram_tensor("TRI", [128, 128], BF16, kind="ExternalInput").ap()
    ONES64 = nc.dram_tensor("ONES64", [1, 64], F32R, kind="ExternalInput").ap()
    OUT = nc.dram_tensor("OUT", [T, D], F32, kind="ExternalOutput").ap()
    if debug:
        DBG_QKT = nc.dram_tensor("DBG_QKT", [128, 4 * T], F32, kind="ExternalOutput").ap()
        DBG_V = nc.dram_tensor("DBG_V", [128, TC128 * 260], BF16, kind="ExternalOutput").ap()
        DBG_ATT = nc.dram_tensor("DBG_ATT", [128, 2 * T], F32, kind="ExternalOutput").ap()

    MUL = mybir.AluOpType.mult
    EXP = mybir.ActivationFunctionType.Exp

    with tile.TileContext(nc) as tc:
        with (
            tc.tile_pool(name="persist", bufs=1) as persist,
            tc.tile_pool(name="p1w", bufs=1) as p1w,
            tc.tile_pool(name="p1t", bufs=3) as p1t,
            tc.tile_pool(name="p2e", bufs=5) as p2e,
            tc.tile_pool(name="p2bc", bufs=2) as p2bc,
            tc.tile_pool(name="p2r", bufs=1) as p2r,
                        tc.tile_pool(name="pj", bufs=2, space="PSUM") as pj,
            tc.tile_pool(name="sps", bufs=3, space="PSUM") as sps,
            tc.tile_pool(name="avps", bufs=3, space="PSUM") as avps,
        ):
            # ---- persistent tiles --------------------------------------
            qkT = persist.tile([128, 4 * T], F32R, tag="qkT")       # Qp0 Kp0 Qp1 Kp1
            v_sb = persist.tile([128, TC128 * 260], BF16, tag="v_sb")  # [jc, head, 64+1]
            attnT = persist.tile([128, 2 * T], F32R, tag="attnT")   # c-chunks x t
            wout_sb = persist.tile([128, 2 * D], F32R, tag="wout_sb")
            tri_sb = persist.tile([128, 128], BF16, tag="tri_sb")

            x_sb = p1w.tile([128, KC * T], F32R, tag="x_sb")
            xlast = p1w.tile([1, T], F32R, tag="xlast")
            wqk_sb = p1w.tile([128, KC * 512], F32R, tag="wqk_sb")
            wqk_last = p1w.tile([1, 512], F32R, tag="wqk_last")
            wv_sb = p1w.tile([128, KC * 256], F32R, tag="wv_sb")
            wv_last = p1w.tile([1, 256], F32R, tag="wv_last")
            psw_sb = p1w.tile([128, 128], F32R, tag="psw_sb")
            cq_sb = p1w.tile([128, T], F32, tag="cq_sb")
            sq_sb = p1w.tile([128, T], F32, tag="sq_sb")

            nc.sync.dma_start(tri_sb[:], TRI[:])
            for k in range(KC):
                nc.sync.dma_start(x_sb[:, k * T:(k + 1) * T], XT[k * 128:(k + 1) * 128, :])
                nc.sync.dma_start(wqk_sb[:, k * 512:(k + 1) * 512], WQK[k * 128:(k + 1) * 128, :])
                nc.sync.dma_start(wv_sb[:, k * 256:(k + 1) * 256], WV[k * 128:(k + 1) * 128, :])
            nc.sync.dma_start(xlast[:], XT[D:D + 1, :])
            nc.sync.dma_start(wqk_last[:], WQK[D:D + 1, :])
            nc.sync.dma_start(wv_last[:], WV[D:D + 1, :])
            nc.sync.dma_start(psw_sb[:], PSW[:])
            nc.sync.dma_start(cq_sb[:], CQ[:])
            nc.sync.dma_start(sq_sb[:], SQ[:])
            for cc in range(2):
                nc.sync.dma_start(wout_sb[:, cc * D:(cc + 1) * D], WOUT[cc * 128:(cc + 1) * 128, :])

            # ones columns of v_aug: one strided memset
            v4 = v_sb[:].rearrange("p (jc h e) -> p jc h e", jc=TC128, h=4)
            nc.vector.memset(v4[:, :, :, 64:65], 1.0)

            # ---------------- building blocks ---------------------------
            def qk_proj_chunk(m, n):
                """project q/k m-chunk (128 channels) for t-chunk n (512), apply rope."""
                is_q = (m % 2 == 0)
                nsl = slice(n * 512, (n + 1) * 512)
                ps = pj.tile([128, 512], F32, tag="pj", name=f"psqk_{m}_{n}")
                for k in range(KC):
                    nc.tensor.matmul(
                        ps[:],
                        wqk_sb[:, k * 512 + m * 128:k * 512 + (m + 1) * 128],
                        x_sb[:, k * T + n * 512:k * T + (n + 1) * 512],
                        start=(k == 0), stop=False,
                    )
                nc.tensor.matmul(
                    ps[:], wqk_last[:, m * 128:(m + 1) * 128], xlast[:, nsl],
                    start=False, stop=True,
                )
                tmp_s = p1t.tile([128, 512], F32R, tag="tmp_s", name=f"tmps_{m}_{n}")
                tmp_c = p1t.tile([128, 512], F32, tag="tmp_c", name=f"tmpc_{m}_{n}")
                if is_q:
                    nc.vector.tensor_mul(tmp_s[:], ps[:], sq_sb[:, nsl])
                    nc.vector.tensor_mul(tmp_c[:], ps[:], cq_sb[:, nsl])
                else:
                    nc.vector.scalar_tensor_tensor(tmp_s[:], ps[:], 8.0, sq_sb[:, nsl], MUL, MUL)
                    nc.vector.scalar_tensor_tensor(tmp_c[:], ps[:], 8.0, cq_sb[:, nsl], MUL, MUL)
                sw = pj.tile([128, 512], F32, tag="pj", name=f"sw_{m}_{n}")
                nc.tensor.matmul(sw[:], psw_sb[:], tmp_s[:], start=True, stop=True)
                nc.vector.tensor_add(qkT[:, m * T + n * 512:m * T + (n + 1) * 512], sw[:], tmp_c[:])

            def v_proj_chunk(tcc):
                tsl = slice(tcc * 128, (tcc + 1) * 128)
                psv = pj.tile([128, 256], F32, tag="pj", name=f"psv_{tcc}")
                for k in range(KC):
                    nc.tensor.matmul(
                        psv[:],
                        x_sb[:, k * T + tcc * 128:k * T + (tcc + 1) * 128],
                        wv_sb[:, k * 256:(k + 1) * 256],
                        start=(k == 0), stop=False,
                    )
                nc.tensor.matmul(psv[:], xlast[:, tsl], wv_last[:], start=False, stop=True)
                vdst = v_sb[:, tcc * 260:(tcc + 1) * 260].rearrange(
                    "p (h e) -> p h e", h=4)[:, :, 0:64]
                vsrc = psv[:].rearrange("p (h e) -> p h e", e=64)
                nc.vector.tensor_copy(vdst, vsrc)

            def attn_ic(p, ic):
                """attention for head-pair p, query chunk ic (512 queries)."""
                qof = (2 * p) * T
                kof = (2 * p + 1) * T
                njc = 4 * ic + 4
                av = [avps.tile([65, 512], F32, tag="av", name=f"av_{p}_{ic}_{i}") for i in range(2)]
                for jc in range(njc):
                    rel = jc - 4 * ic
                    ls = 0 if rel < 0 else rel * 128
                    e_t = [p2e.tile([128, 512], BF16, tag="e_t", name=f"e_{p}_{ic}_{jc}_{i}") for i in range(2)]
                    for hh in range(2):
                        pof = hh * 64
                        s_ps = sps.tile([128, 512], F32, tag="s_ps", name=f"s_{p}_{ic}_{jc}_{hh}")
                        nc.tensor.matmul(
                            s_ps[:],
                            qkT[pof:pof + 64, kof + jc * 128:kof + (jc + 1) * 128],
                            qkT[pof:pof + 64, qof + ic * 512:qof + (ic + 1) * 512],
                            start=True, stop=True,
                        )
                        if ls > 0:
                            nc.gpsimd.memset(e_t[hh][:, 0:ls], 0.0)
                        nc.scalar.activation(e_t[hh][:, ls:512], s_ps[:, ls:512], EXP)
                        if rel >= 0:
                            tri_slice = slice(rel * 128, (rel + 1) * 128)
                            nc.vector.tensor_mul(e_t[hh][:, tri_slice], e_t[hh][:, tri_slice], tri_sb[:])
                    for hh in range(2):
                        nc.tensor.matmul(
                            av[hh][:],
                            v_sb[:, jc * 260 + (2 * p + hh) * 65:jc * 260 + (2 * p + hh) * 65 + 65],
                            e_t[hh][:],
                            start=(jc == 0), stop=(jc == njc - 1),
                            skip_group_check=True,
                        )
                for hh in range(2):
                    head = 2 * p + hh
                    den = p2r.tile([1, 512], F32, tag="den", name=f"den_{p}_{ic}_{hh}")
                    nc.vector.tensor_copy(den[:], av[hh][64:65, :])
                    rec = p2r.tile([1, 512], F32, tag="rec", name=f"rec_{p}_{ic}_{hh}")
                    rsc = p2r.tile([1, 512], F32, tag="rsc", name=f"rsc_{p}_{ic}_{hh}")
                    nc.vector.reciprocal_approx_accurate(rec[:], den[:], rsc[:])
                    bc_sb = p2bc.tile([64, 512], F32, tag="bc_sb", name=f"bc_{p}_{ic}_{hh}")
                    nc.gpsimd.partition_broadcast(bc_sb[:], rec[:], channels=64)
                    cof = (head // 2) * T
                    pof = (head % 2) * 64
                    nc.vector.tensor_mul(
                        attnT[pof:pof + 64, cof + ic * 512:cof + (ic + 1) * 512],
                        av[hh][0:64, :], bc_sb[:],
                    )

            def out_proj_chunk(tcc):
                tsl = slice(tcc * 128, (tcc + 1) * 128)
                pos = [pj.tile([128, 512], F32, tag="pj", name=f"po_{tcc}_{oc}") for oc in range(2)]
                for cc in range(2):
                    for oc in range(2):
                        nc.tensor.matmul(
                            pos[oc][:],
                            attnT[:, cc * T + tcc * 128:cc * T + (tcc + 1) * 128],
                            wout_sb[:, cc * D + oc * 512:cc * D + (oc + 1) * 512],
                            start=(cc == 0), stop=(cc == 1),
                        )
                for oc in range(2):
                    osl = slice(oc * 512, (oc + 1) * 512)
                    po_sb = p1t.tile([128, 512], F32, tag="tmp_c", name=f"po_sb_{tcc}_{oc}")
                    if oc == 0:
                        nc.vector.tensor_copy(po_sb[:], pos[oc][:])
                    else:
                        nc.scalar.copy(po_sb[:], pos[oc][:])
                    nc.sync.dma_start(OUT[tsl, osl], po_sb[:])

            # ---------------- schedule ----------------------------------
            # pair-0 q/k projection
            for m in range(2):
                for n in range(TC512):
                    qk_proj_chunk(m, n)
            # pair-0 attention, interleaved with v-projection (staggered so
            # v chunks jc<=4ic+3 land before attn ic needs them) and with
            # pair-1 q/k projection (PE filler to keep HAM warm).
            for ic in range(TC512):
                for tcc in range(4 * ic, 4 * ic + 4):
                    v_proj_chunk(tcc)
                for m in (2, 3):
                    qk_proj_chunk(m, ic)
                attn_ic(0, ic)
            # pair-1 attention interleaved with output projection of t-chunks
            # whose attnT columns are complete (both pairs done through ic).
            for ic in range(TC512):
                attn_ic(1, ic)
                for tcc in range(4 * ic, 4 * ic + 4):
                    out_proj_chunk(tcc)

            if debug:
                nc.sync.dma_start(DBG_QKT[:], qkT[:].bitcast(F32))
                nc.sync.dma_start(DBG_V[:], v_sb[:])
                nc.sync.dma_start(DBG_ATT[:], attnT[:].bitcast(F32))

    nc.compile()
    return nc


_DEINT = list(range(0, DK, 2)) + list(range(1, DK, 2))


def _rope_tables():
    j = np.arange(DK // 2, dtype=np.float64)
    inv_freq = THETA ** (-2.0 * j / DK)
    t = np.arange(T, dtype=np.float64)
    ang = t[None, :] * inv_freq[:, None]          # [32, T]
    ang = np.tile(ang, (4, 1))                    # [128, T]
    return np.cos(ang).astype(np.float32), np.sin(ang).astype(np.float32)


def _psw():
    M = np.zeros((128, 128), dtype=np.float32)
    for p in range(128):
        pm = p % 64
        if pm < 32:
            M[p, p + 32] = -1.0
        else:
            M[p, p - 32] = 1.0
    return np.ascontiguousarray(M.T)


def shard_inputs(x, Wqkv, bqkv, Wout, bout):
    x = np.asarray(x, dtype=np.float32)
    Wqkv = np.asarray(Wqkv, dtype=np.float32)
    bqkv = np.asarray(bqkv, dtype=np.float32)
    Wout = np.asarray(Wout, dtype=np.float32)

    cos_t, sin_t = _rope_tables()
    cq = np.ascontiguousarray(cos_t / 8.0)
    sq = np.ascontiguousarray(sin_t / 8.0)
    psw = _psw()
    tri = np.triu(np.ones((128, 128), dtype=np.float32)).astype(ml_dtypes.bfloat16)
    ones64 = np.ones((1, 64), dtype=np.float32)

    Wfull = np.concatenate([Wqkv, bqkv[:, None]], axis=1)  # [3072, 1025]

    xt = {}
    for b in range(B):
        xt[b] = np.ascontiguousarray(
            np.concatenate([x[b].T, np.ones((1, T), np.float32)], axis=0)
        )

    in_maps = []
    for c in range(NCORES):
        b = c // 4
        heads = [4 * (c % 4) + i for i in range(HEADS_PER_CORE)]
        # chunk order: [Qp0 | Kp0 | Qp1 | Kp1], each 128 rows (2 heads x 64)
        qk_rows = []
        for p in range(2):
            qrows, krows = [], []
            for h in (2 * p, 2 * p + 1):
                H = heads[h]
                qrows += [H * 192 + j for j in _DEINT]
                krows += [H * 192 + 64 + j for j in _DEINT]
            qk_rows += qrows + krows
        v_rows = []
        for h in range(4):
            H = heads[h]
            v_rows += [H * 192 + 128 + j for j in range(DK)]
        vch_out = []
        for h in range(4):
            H = heads[h]
            vch_out += [H * 64 + j for j in range(DK)]

        in_maps.append({
            "XT": xt[b],
            "WQK": np.ascontiguousarray(Wfull[qk_rows].T),
            "WV": np.ascontiguousarray(Wfull[v_rows].T),
            "WOUT": np.ascontiguousarray(Wout[:, vch_out].T),
            "PSW": psw,
            "CQ": cq,
            "SQ": sq,
            "TRI": tri,
            "ONES64": ones64,
        })
    return in_maps


_CACHED = {}


def _get_program(debug=False):
    key = bool(debug)
    if key not in _CACHED:
        _CACHED[key] = build_program(debug=debug)
    return _CACHED[key]


def run_cores(inputs, debug=False, trace=False, tmpdir=None):
    nc = _get_program(debug=debug)
    in_maps = shard_inputs(**inputs)
    res = run_bass_kernel_spmd(
        nc, in_maps, core_ids=list(range(NCORES)), trace=trace, tmpdir=tmpdir,
    )
    return res


def combine(results, bout):
    bout = np.asarray(bout, dtype=np.float32)
    out = np.empty((B, T, D), dtype=np.float32)
    for b in range(B):
        acc = results[4 * b]["OUT"].astype(np.float32).copy()
        for c in range(4 * b + 1, 4 * b + 4):
            acc += results[c]["OUT"]
        out[b] = acc + bout[None, :]
    return out


def kernel(x, Wqkv, bqkv, Wout, bout):
    res = run_cores(dict(x=x, Wqkv=Wqkv, bqkv=bqkv, Wout=Wout, bout=bout))
    return combine(res.results, bout)


# revision 14
# speedup vs baseline: 1.2827x; 1.0210x over previous
"""Causal self-attention with rotary embeddings (B=2, T=2048, D=1024, H=16,
d_k=64) on 8 Trainium2 NeuronCores.

Sharding: core c handles batch b = c//4 and 4 heads (c%4)*4..+4 — data
parallel on B, tensor parallel on heads.  Each core computes its heads'
qkv projection, RoPE, causal attention, and a partial output projection
over its 256 attention channels; the host sums the 4 partials per batch.

Layout tricks:
  * q/k channels are de-interleaved host-side (RoPE pair -> half-split
    form) and packed 2 heads per 128-partition tile; scores matmuls are
    row-tiled K=64 pairs.
  * RoPE swap (+/- sign) is a 128x128 permutation matmul on TensorE; the
    cos/sin elementwise work runs on VectorE fused with PSUM eviction.
  * softmax skips max-subtraction (scores ~ N(0,1), bounded) and folds the
    denominator into attn@v as an extra ones-column of v; the divide is a
    per-head broadcast-reciprocal multiply at eviction.
  * all matmul inputs are float32r (TF32-rate on TensorE at full fp32
    memory layout); exp outputs / v are bf16.
"""

import sys

sys.path.insert(0, "/opt/trn_rl_repo")

import numpy as np
import ml_dtypes

import concourse.bacc as bacc
import concourse.tile as tile
from concourse import mybir
from concourse.bass_utils import run_bass_kernel_spmd

F32 = mybir.dt.float32
F32R = mybir.dt.float32r
BF16 = mybir.dt.bfloat16

B, T, D = 2, 2048, 1024
NH, DK = 16, 64
THETA = 10000.0
NCORES = 8
HEADS_PER_CORE = 4

TC512 = T // 512        # 4   i-chunks of 512
TC128 = T // 128        # 16  t/j-chunks of 128
KC = D // 128           # 8   d_model contraction chunks


def build_program(debug=False):
    nc = bacc.Bacc("TRN2", target_bir_lowering=False, debug=False)

    XT = nc.dram_tensor("XT", [D + 1, T], F32R, kind="ExternalInput").ap()
    WQK = nc.dram_tensor("WQK", [D + 1, 512], F32R, kind="ExternalInput").ap()
    WV = nc.dram_tensor("WV", [D + 1, 256], F32R, kind="ExternalInput").ap()
    WOUT = nc.dram_tensor("WOUT", [256, D], F32R, kind="ExternalInput").ap()
    PSW = nc.dram_tensor("PSW", [128, 128], F32R, kind="ExternalInput").ap()
    CQ = nc.dram_tensor("CQ", [128, T], F32, kind="ExternalInput").ap()
    SQ = nc.dram_tensor("SQ", [128, T], F32, kind="ExternalInput").ap()
    TRI = nc.dram_tensor("TRI", [128, 128], BF16, kind="ExternalInput").ap()
    ONES64 = nc.dram_tensor("ONES64", [1, 64], F32R, kind="ExternalInput").ap()
    OUT = nc.dram_tensor("OUT", [T, D], F32, kind="ExternalOutput").ap()
    if debug:
        DBG_QKT = nc.dram_tensor("DBG_QKT", [128, 4 * T], F32, kind="ExternalOutput").ap()
        DBG_V = nc.dram_tensor("DBG_V", [128, TC128 * 260], BF16, kind="ExternalOutput").ap()
        DBG_ATT = nc.dram_tensor("DBG_ATT", [128, 2 * T], F32, kind="ExternalOutput").ap()

    MUL = mybir.AluOpType.mult
    EXP = mybir.ActivationFunctionType.Exp

    with tile.TileContext(nc) as tc:
        with (
            tc.tile_pool(name="persist", bufs=1) as persist,
            tc.tile_pool(name="p1w", bufs=1) as p1w,
            tc.tile_pool(name="p1t", bufs=3) as p1t,
            tc.tile_pool(name="p2e", bufs=5) as p2e,
            tc.tile_pool(name="p2bc", bufs=2) as p2bc,
            tc.tile_pool(name="p2r", bufs=1) as p2r,
                        tc.tile_pool(name="pj", bufs=2, space="PSUM") as pj,
            tc.tile_pool(name="sps", bufs=3, space="PSUM") as sps,
            tc.tile_pool(name="avps", bufs=3, space="PSUM") as avps,
        ):
            # ---- persistent tiles --------------------------------------
            qkT = persist.tile([128, 4 * T], F32R, tag="qkT")       # Qp0 Kp0 Qp1 Kp1
            v_sb = persist.tile([128, TC128 * 260], BF16, tag="v_sb")  # [jc, head, 64+1]
            attnT = persist.tile([128, 2 * T], F32R, tag="attnT")   # c-chunks x t
            wout_sb = persist.tile([128, 2 * D], F32R, tag="wout_sb")
            tri_sb = persist.tile([128, 128], BF16, tag="tri_sb")

            x_sb = p1w.tile([128, KC * T], F32R, tag="x_sb")
            xlast = p1w.tile([1, T], F32R, tag="xlast")
            wqk_sb = p1w.tile([128, KC * 512], F32R, tag="wqk_sb")
            wqk_last = p1w.tile([1, 512], F32R, tag="wqk_last")
            wv_sb = p1w.tile([128, KC * 256], F32R, tag="wv_sb")
            wv_last = p1w.tile([1, 256], F32R, tag="wv_last")
            psw_sb = p1w.tile([128, 128], F32R, tag="psw_sb")
            cq_sb = p1w.tile([128, T], F32, tag="cq_sb")
            sq_sb = p1w.tile([128, T], F32, tag="sq_sb")

            nc.sync.dma_start(tri_sb[:], TRI[:])
            for k in range(KC):
                nc.sync.dma_start(wqk_sb[:, k * 512:(k + 1) * 512], WQK[k * 128:(k + 1) * 128, :])
            for n in range(TC512):
                for k in range(KC):
                    nc.sync.dma_start(
                        x_sb[:, k * T + n * 512:k * T + (n + 1) * 512],
                        XT[k * 128:(k + 1) * 128, n * 512:(n + 1) * 512])
            for k in range(KC):
                nc.sync.dma_start(wv_sb[:, k * 256:(k + 1) * 256], WV[k * 128:(k + 1) * 128, :])
            nc.sync.dma_start(xlast[:], XT[D:D + 1, :])
            nc.sync.dma_start(wqk_last[:], WQK[D:D + 1, :])
            nc.sync.dma_start(wv_last[:], WV[D:D + 1, :])
            nc.sync.dma_start(psw_sb[:], PSW[:])
            nc.sync.dma_start(cq_sb[:], CQ[:])
            nc.sync.dma_start(sq_sb[:], SQ[:])
            for cc in range(2):
                nc.sync.dma_start(wout_sb[:, cc * D:(cc + 1) * D], WOUT[cc * 128:(cc + 1) * 128, :])

            # ones columns of v_aug: one strided memset
            v4 = v_sb[:].rearrange("p (jc h e) -> p jc h e", jc=TC128, h=4)
            nc.vector.memset(v4[:, :, :, 64:65], 1.0)

            # ---------------- building blocks ---------------------------
            def qk_proj_chunk(m, n):
                """project q/k m-chunk (128 channels) for t-chunk n (512), apply rope."""
                is_q = (m % 2 == 0)
                nsl = slice(n * 512, (n + 1) * 512)
                ps = pj.tile([128, 512], F32, tag="pj", name=f"psqk_{m}_{n}")
                for k in range(KC):
                    nc.tensor.matmul(
                        ps[:],
                        wqk_sb[:, k * 512 + m * 128:k * 512 + (m + 1) * 128],
                        x_sb[:, k * T + n * 512:k * T + (n + 1) * 512],
                        start=(k == 0), stop=False,
                    )
                nc.tensor.matmul(
                    ps[:], wqk_last[:, m * 128:(m + 1) * 128], xlast[:, nsl],
                    start=False, stop=True,
                )
                tmp_s = p1t.tile([128, 512], F32R, tag="tmp_s", name=f"tmps_{m}_{n}")
                tmp_c = p1t.tile([128, 512], F32, tag="tmp_c", name=f"tmpc_{m}_{n}")
                if is_q:
                    nc.vector.tensor_mul(tmp_s[:], ps[:], sq_sb[:, nsl])
                    nc.vector.tensor_mul(tmp_c[:], ps[:], cq_sb[:, nsl])
                else:
                    nc.vector.scalar_tensor_tensor(tmp_s[:], ps[:], 8.0, sq_sb[:, nsl], MUL, MUL)
                    nc.vector.scalar_tensor_tensor(tmp_c[:], ps[:], 8.0, cq_sb[:, nsl], MUL, MUL)
                sw = pj.tile([128, 512], F32, tag="pj", name=f"sw_{m}_{n}")
                nc.tensor.matmul(sw[:], psw_sb[:], tmp_s[:], start=True, stop=True)
                nc.vector.tensor_add(qkT[:, m * T + n * 512:m * T + (n + 1) * 512], sw[:], tmp_c[:])

            def v_proj_chunk(tcc):
                tsl = slice(tcc * 128, (tcc + 1) * 128)
                psv = pj.tile([128, 256], F32, tag="pj", name=f"psv_{tcc}")
                for k in range(KC):
                    nc.tensor.matmul(
                        psv[:],
                        x_sb[:, k * T + tcc * 128:k * T + (tcc + 1) * 128],
                        wv_sb[:, k * 256:(k + 1) * 256],
                        start=(k == 0), stop=False,
                    )
                nc.tensor.matmul(psv[:], xlast[:, tsl], wv_last[:], start=False, stop=True)
                vdst = v_sb[:, tcc * 260:(tcc + 1) * 260].rearrange(
                    "p (h e) -> p h e", h=4)[:, :, 0:64]
                vsrc = psv[:].rearrange("p (h e) -> p h e", e=64)
                nc.vector.tensor_copy(vdst, vsrc)

            def attn_ic(p, ic, fillers=()):
                """attention for head-pair p, query chunk ic (512 queries).
                fillers: callables run one per jc iteration (PE density)."""
                fillers = list(fillers)
                qof = (2 * p) * T
                kof = (2 * p + 1) * T
                njc = 4 * ic + 4
                av = [avps.tile([65, 512], F32, tag="av", name=f"av_{p}_{ic}_{i}") for i in range(2)]
                for jc in range(njc):
                    rel = jc - 4 * ic
                    ls = 0 if rel < 0 else rel * 128
                    e_t = [p2e.tile([128, 512], BF16, tag="e_t", name=f"e_{p}_{ic}_{jc}_{i}") for i in range(2)]
                    for hh in range(2):
                        pof = hh * 64
                        s_ps = sps.tile([128, 512], F32, tag="s_ps", name=f"s_{p}_{ic}_{jc}_{hh}")
                        nc.tensor.matmul(
                            s_ps[:],
                            qkT[pof:pof + 64, kof + jc * 128:kof + (jc + 1) * 128],
                            qkT[pof:pof + 64, qof + ic * 512:qof + (ic + 1) * 512],
                            start=True, stop=True,
                        )
                        if ls > 0:
                            nc.gpsimd.memset(e_t[hh][:, 0:ls], 0.0)
                        nc.scalar.activation(e_t[hh][:, ls:512], s_ps[:, ls:512], EXP)
                        if rel >= 0:
                            tri_slice = slice(rel * 128, (rel + 1) * 128)
                            nc.vector.tensor_mul(e_t[hh][:, tri_slice], e_t[hh][:, tri_slice], tri_sb[:])
                    for hh in range(2):
                        nc.tensor.matmul(
                            av[hh][:],
                            v_sb[:, jc * 260 + (2 * p + hh) * 65:jc * 260 + (2 * p + hh) * 65 + 65],
                            e_t[hh][:],
                            start=(jc == 0), stop=(jc == njc - 1),
                            skip_group_check=True,
                        )
                    if fillers and (jc % max(1, njc // len(fillers)) == 0 or jc == njc - 1):
                        while fillers and len(fillers) > (njc - 1 - jc):
                            fillers.pop(0)()
                for hh in range(2):
                    head = 2 * p + hh
                    den = p2r.tile([1, 512], F32, tag="den", name=f"den_{p}_{ic}_{hh}")
                    nc.vector.tensor_copy(den[:], av[hh][64:65, :])
                    rec = p2r.tile([1, 512], F32, tag="rec", name=f"rec_{p}_{ic}_{hh}")
                    rsc = p2r.tile([1, 512], F32, tag="rsc", name=f"rsc_{p}_{ic}_{hh}")
                    nc.vector.reciprocal_approx_accurate(rec[:], den[:], rsc[:])
                    bc_sb = p2bc.tile([64, 512], F32, tag="bc_sb", name=f"bc_{p}_{ic}_{hh}")
                    nc.gpsimd.partition_broadcast(bc_sb[:], rec[:], channels=64)
                    cof = (head // 2) * T
                    pof = (head % 2) * 64
                    nc.vector.tensor_mul(
                        attnT[pof:pof + 64, cof + ic * 512:cof + (ic + 1) * 512],
                        av[hh][0:64, :], bc_sb[:],
                    )

            def out_proj_chunk(tcc):
                tsl = slice(tcc * 128, (tcc + 1) * 128)
                for oc in range(2):
                    po = pj.tile([128, 512], F32, tag="pj", name=f"po_{tcc}_{oc}")
                    for cc in range(2):
                        nc.tensor.matmul(
                            po[:],
                            attnT[:, cc * T + tcc * 128:cc * T + (tcc + 1) * 128],
                            wout_sb[:, cc * D + oc * 512:cc * D + (oc + 1) * 512],
                            start=(cc == 0), stop=(cc == 1),
                        )
                    osl = slice(oc * 512, (oc + 1) * 512)
                    po_sb = p1t.tile([128, 512], F32, tag="tmp_c", name=f"po_sb_{tcc}_{oc}")
                    if oc == 0:
                        nc.vector.tensor_copy(po_sb[:], po[:])
                    else:
                        nc.scalar.copy(po_sb[:], po[:])
                    nc.sync.dma_start(OUT[tsl, osl], po_sb[:])

            # ---------------- schedule ----------------------------------
            # pair-0 q/k projection, then v chunks needed by attn(0, ic=0)
            for m in range(2):
                for n in range(TC512):
                    qk_proj_chunk(m, n)
            for tcc in range(4):
                v_proj_chunk(tcc)
            # pair-0 attention with interleaved fillers: v chunks for the
            # next ic, and pair-1 q/k projection chunks.
            for ic in range(TC512):
                fillers = []
                if ic < 3:
                    fillers += [(lambda t=t: v_proj_chunk(t)) for t in range(4 * ic + 4, 4 * ic + 8)]
                fillers += [(lambda m=m: qk_proj_chunk(m, ic)) for m in (2, 3)]
                attn_ic(0, ic, fillers)
            # pair-1 attention with out-proj of completed t-chunks as filler
            for ic in range(TC512):
                fillers = []
                if ic >= 1:
                    fillers += [(lambda t=t: out_proj_chunk(t)) for t in range(4 * (ic - 1), 4 * (ic - 1) + 4)]
                attn_ic(1, ic, fillers)
            for tcc in range(12, 16):
                out_proj_chunk(tcc)

            if debug:
                nc.sync.dma_start(DBG_QKT[:], qkT[:].bitcast(F32))
                nc.sync.dma_start(DBG_V[:], v_sb[:])
                nc.sync.dma_start(DBG_ATT[:], attnT[:].bitcast(F32))

    nc.compile()
    return nc


_DEINT = list(range(0, DK, 2)) + list(range(1, DK, 2))


def _rope_tables():
    j = np.arange(DK // 2, dtype=np.float64)
    inv_freq = THETA ** (-2.0 * j / DK)
    t = np.arange(T, dtype=np.float64)
    ang = t[None, :] * inv_freq[:, None]          # [32, T]
    ang = np.tile(ang, (4, 1))                    # [128, T]
    return np.cos(ang).astype(np.float32), np.sin(ang).astype(np.float32)


def _psw():
    M = np.zeros((128, 128), dtype=np.float32)
    for p in range(128):
        pm = p % 64
        if pm < 32:
            M[p, p + 32] = -1.0
        else:
            M[p, p - 32] = 1.0
    return np.ascontiguousarray(M.T)


def shard_inputs(x, Wqkv, bqkv, Wout, bout):
    x = np.asarray(x, dtype=np.float32)
    Wqkv = np.asarray(Wqkv, dtype=np.float32)
    bqkv = np.asarray(bqkv, dtype=np.float32)
    Wout = np.asarray(Wout, dtype=np.float32)

    cos_t, sin_t = _rope_tables()
    cq = np.ascontiguousarray(cos_t / 8.0)
    sq = np.ascontiguousarray(sin_t / 8.0)
    psw = _psw()
    tri = np.triu(np.ones((128, 128), dtype=np.float32)).astype(ml_dtypes.bfloat16)
    ones64 = np.ones((1, 64), dtype=np.float32)

    Wfull = np.concatenate([Wqkv, bqkv[:, None]], axis=1)  # [3072, 1025]

    xt = {}
    for b in range(B):
        xt[b] = np.ascontiguousarray(
            np.concatenate([x[b].T, np.ones((1, T), np.float32)], axis=0)
        )

    in_maps = []
    for c in range(NCORES):
        b = c // 4
        heads = [4 * (c % 4) + i for i in range(HEADS_PER_CORE)]
        # chunk order: [Qp0 | Kp0 | Qp1 | Kp1], each 128 rows (2 heads x 64)
        qk_rows = []
        for p in range(2):
            qrows, krows = [], []
            for h in (2 * p, 2 * p + 1):
                H = heads[h]
                qrows += [H * 192 + j for j in _DEINT]
                krows += [H * 192 + 64 + j for j in _DEINT]
            qk_rows += qrows + krows
        v_rows = []
        for h in range(4):
            H = heads[h]
            v_rows += [H * 192 + 128 + j for j in range(DK)]
        vch_out = []
        for h in range(4):
            H = heads[h]
            vch_out += [H * 64 + j for j in range(DK)]

        in_maps.append({
            "XT": xt[b],
            "WQK": np.ascontiguousarray(Wfull[qk_rows].T),
            "WV": np.ascontiguousarray(Wfull[v_rows].T),
            "WOUT": np.ascontiguousarray(Wout[:, vch_out].T),
            "PSW": psw,
            "CQ": cq,
            "SQ": sq,
            "TRI": tri,
            "ONES64": ones64,
        })
    return in_maps


_CACHED = {}


def _get_program(debug=False):
    key = bool(debug)
    if key not in _CACHED:
        _CACHED[key] = build_program(debug=debug)
    return _CACHED[key]


def run_cores(inputs, debug=False, trace=False, tmpdir=None):
    nc = _get_program(debug=debug)
    in_maps = shard_inputs(**inputs)
    res = run_bass_kernel_spmd(
        nc, in_maps, core_ids=list(range(NCORES)), trace=trace, tmpdir=tmpdir,
    )
    return res


def combine(results, bout):
    bout = np.asarray(bout, dtype=np.float32)
    out = np.empty((B, T, D), dtype=np.float32)
    for b in range(B):
        acc = results[4 * b]["OUT"].astype(np.float32).copy()
        for c in range(4 * b + 1, 4 * b + 4):
            acc += results[c]["OUT"]
        out[b] = acc + bout[None, :]
    return out


def kernel(x, Wqkv, bqkv, Wout, bout):
    res = run_cores(dict(x=x, Wqkv=Wqkv, bqkv=bqkv, Wout=Wout, bout=bout))
    return combine(res.results, bout)


# revision 15
# speedup vs baseline: 1.4007x; 1.0920x over previous
"""Causal self-attention with rotary embeddings (B=2, T=2048, D=1024, H=16,
d_k=64) on 8 Trainium2 NeuronCores.

Sharding: core c handles batch b = c//4 and 4 heads (c%4)*4..+4 — data
parallel on B, tensor parallel on heads.  Each core computes its heads'
qkv projection, RoPE, causal attention, and a partial output projection
over its 256 attention channels; the host sums the 4 partials per batch.

Layout tricks:
  * q/k channels are de-interleaved host-side (RoPE pair -> half-split
    form) and packed 2 heads per 128-partition tile; scores matmuls are
    row-tiled K=64 pairs.
  * RoPE swap (+/- sign) is a 128x128 permutation matmul on TensorE; the
    cos/sin elementwise work runs on VectorE fused with PSUM eviction.
  * softmax skips max-subtraction (scores ~ N(0,1), bounded) and folds the
    denominator into attn@v as an extra ones-column of v; the divide is a
    per-head broadcast-reciprocal multiply at eviction.
  * all matmul inputs are float32r (TF32-rate on TensorE at full fp32
    memory layout); exp outputs / v are bf16.
"""

import sys

sys.path.insert(0, "/opt/trn_rl_repo")

import numpy as np
import ml_dtypes

import concourse.bacc as bacc
import concourse.tile as tile
from concourse import mybir
from concourse.bass_utils import run_bass_kernel_spmd

F32 = mybir.dt.float32
F32R = mybir.dt.float32r
BF16 = mybir.dt.bfloat16

B, T, D = 2, 2048, 1024
NH, DK = 16, 64
THETA = 10000.0
NCORES = 8
HEADS_PER_CORE = 4

TC512 = T // 512        # 4   i-chunks of 512
TC128 = T // 128        # 16  t/j-chunks of 128
KC = D // 128           # 8   d_model contraction chunks


def build_program(debug=False):
    nc = bacc.Bacc("TRN2", target_bir_lowering=False, debug=False)

    XT = nc.dram_tensor("XT", [D + 1, T], F32R, kind="ExternalInput").ap()
    WQK = nc.dram_tensor("WQK", [D + 1, 512], F32R, kind="ExternalInput").ap()
    WV = nc.dram_tensor("WV", [D + 1, 256], F32R, kind="ExternalInput").ap()
    WOUT = nc.dram_tensor("WOUT", [256, D], F32R, kind="ExternalInput").ap()
    PSW = nc.dram_tensor("PSW", [128, 128], F32R, kind="ExternalInput").ap()
    CQ = nc.dram_tensor("CQ", [128, T], F32, kind="ExternalInput").ap()
    SQ = nc.dram_tensor("SQ", [128, T], F32, kind="ExternalInput").ap()
    TRI = nc.dram_tensor("TRI", [128, 128], BF16, kind="ExternalInput").ap()
    ONES64 = nc.dram_tensor("ONES64", [1, 64], F32R, kind="ExternalInput").ap()
    OUT = nc.dram_tensor("OUT", [T, D], F32, kind="ExternalOutput").ap()
    if debug:
        DBG_QKT = nc.dram_tensor("DBG_QKT", [128, 4 * T], F32, kind="ExternalOutput").ap()
        DBG_V = nc.dram_tensor("DBG_V", [128, TC128 * 260], BF16, kind="ExternalOutput").ap()
        DBG_ATT = nc.dram_tensor("DBG_ATT", [128, 2 * T], F32, kind="ExternalOutput").ap()

    MUL = mybir.AluOpType.mult
    EXP = mybir.ActivationFunctionType.Exp

    with tile.TileContext(nc) as tc:
        with (
            tc.tile_pool(name="persist", bufs=1) as persist,
            tc.tile_pool(name="p1w", bufs=1) as p1w,
            tc.tile_pool(name="p1t", bufs=3) as p1t,
            tc.tile_pool(name="p2e", bufs=5) as p2e,
            tc.tile_pool(name="p2bc", bufs=2) as p2bc,
            tc.tile_pool(name="p2r", bufs=1) as p2r,
                        tc.tile_pool(name="pj", bufs=2, space="PSUM") as pj,
            tc.tile_pool(name="sps", bufs=3, space="PSUM") as sps,
            tc.tile_pool(name="avps", bufs=3, space="PSUM") as avps,
        ):
            # ---- persistent tiles --------------------------------------
            qkT = persist.tile([128, 4 * T], F32R, tag="qkT")       # Qp0 Kp0 Qp1 Kp1
            v_sb = persist.tile([128, TC128 * 260], BF16, tag="v_sb")  # [jc, head, 64+1]
            attnT = persist.tile([128, 2 * T], F32R, tag="attnT")   # c-chunks x t
            wout_sb = persist.tile([128, 2 * D], F32R, tag="wout_sb")
            tri_sb = persist.tile([128, 128], BF16, tag="tri_sb")

            x_sb = p1w.tile([128, KC * T], F32R, tag="x_sb")
            xlast = p1w.tile([1, T], F32R, tag="xlast")
            wqk_sb = p1w.tile([128, KC * 512], F32R, tag="wqk_sb")
            wqk_last = p1w.tile([1, 512], F32R, tag="wqk_last")
            wv_sb = p1w.tile([128, KC * 256], F32R, tag="wv_sb")
            wv_last = p1w.tile([1, 256], F32R, tag="wv_last")
            psw_sb = p1w.tile([128, 128], F32R, tag="psw_sb")
            cq_sb = p1w.tile([128, T], F32, tag="cq_sb")
            sq_sb = p1w.tile([128, T], F32, tag="sq_sb")

            for k in range(KC):
                nc.sync.dma_start(wqk_sb[:, k * 512:(k + 1) * 512], WQK[k * 128:(k + 1) * 128, :])
            nc.sync.dma_start(wqk_last[:], WQK[D:D + 1, :])
            nc.sync.dma_start(xlast[:], XT[D:D + 1, :])

            def load_x_block(n):
                for k in range(KC):
                    nc.sync.dma_start(
                        x_sb[:, k * T + n * 512:k * T + (n + 1) * 512],
                        XT[k * 128:(k + 1) * 128, n * 512:(n + 1) * 512])

            load_x_block(0)
            nc.sync.dma_start(psw_sb[:], PSW[:])
            nc.sync.dma_start(cq_sb[:], CQ[:])
            nc.sync.dma_start(sq_sb[:], SQ[:])
            nc.sync.dma_start(tri_sb[:], TRI[:])
            for k in range(KC):
                nc.sync.dma_start(wv_sb[:, k * 256:(k + 1) * 256], WV[k * 128:(k + 1) * 128, :])
            nc.sync.dma_start(wv_last[:], WV[D:D + 1, :])
            for n in range(1, TC512):
                load_x_block(n)
            for cc in range(2):
                nc.sync.dma_start(wout_sb[:, cc * D:(cc + 1) * D], WOUT[cc * 128:(cc + 1) * 128, :])

            # ones columns of v_aug: one strided memset
            v4 = v_sb[:].rearrange("p (jc h e) -> p jc h e", jc=TC128, h=4)
            nc.vector.memset(v4[:, :, :, 64:65], 1.0)

            # ---------------- building blocks ---------------------------
            def qk_proj_chunk(m, n):
                """project q/k m-chunk (128 channels) for t-chunk n (512), apply rope."""
                is_q = (m % 2 == 0)
                nsl = slice(n * 512, (n + 1) * 512)
                ps = pj.tile([128, 512], F32, tag="pj", name=f"psqk_{m}_{n}")
                for k in range(KC):
                    nc.tensor.matmul(
                        ps[:],
                        wqk_sb[:, k * 512 + m * 128:k * 512 + (m + 1) * 128],
                        x_sb[:, k * T + n * 512:k * T + (n + 1) * 512],
                        start=(k == 0), stop=False,
                    )
                nc.tensor.matmul(
                    ps[:], wqk_last[:, m * 128:(m + 1) * 128], xlast[:, nsl],
                    start=False, stop=True,
                )
                tmp_s = p1t.tile([128, 512], F32R, tag="tmp_s", name=f"tmps_{m}_{n}")
                tmp_c = p1t.tile([128, 512], F32, tag="tmp_c", name=f"tmpc_{m}_{n}")
                if is_q:
                    nc.vector.tensor_mul(tmp_s[:], ps[:], sq_sb[:, nsl])
                    nc.vector.tensor_mul(tmp_c[:], ps[:], cq_sb[:, nsl])
                else:
                    nc.vector.scalar_tensor_tensor(tmp_s[:], ps[:], 8.0, sq_sb[:, nsl], MUL, MUL)
                    nc.vector.scalar_tensor_tensor(tmp_c[:], ps[:], 8.0, cq_sb[:, nsl], MUL, MUL)
                sw = pj.tile([128, 512], F32, tag="pj", name=f"sw_{m}_{n}")
                nc.tensor.matmul(sw[:], psw_sb[:], tmp_s[:], start=True, stop=True)
                nc.vector.tensor_add(qkT[:, m * T + n * 512:m * T + (n + 1) * 512], sw[:], tmp_c[:])

            def v_proj_chunk(tcc):
                tsl = slice(tcc * 128, (tcc + 1) * 128)
                psv = pj.tile([128, 256], F32, tag="pj", name=f"psv_{tcc}")
                for k in range(KC):
                    nc.tensor.matmul(
                        psv[:],
                        x_sb[:, k * T + tcc * 128:k * T + (tcc + 1) * 128],
                        wv_sb[:, k * 256:(k + 1) * 256],
                        start=(k == 0), stop=False,
                    )
                nc.tensor.matmul(psv[:], xlast[:, tsl], wv_last[:], start=False, stop=True)
                vdst = v_sb[:, tcc * 260:(tcc + 1) * 260].rearrange(
                    "p (h e) -> p h e", h=4)[:, :, 0:64]
                vsrc = psv[:].rearrange("p (h e) -> p h e", e=64)
                nc.vector.tensor_copy(vdst, vsrc)

            def attn_ic(p, ic, fillers=()):
                """attention for head-pair p, query chunk ic (512 queries).
                fillers: callables run one per jc iteration (PE density)."""
                fillers = list(fillers)
                qof = (2 * p) * T
                kof = (2 * p + 1) * T
                njc = 4 * ic + 4
                av = [avps.tile([65, 512], F32, tag="av", name=f"av_{p}_{ic}_{i}") for i in range(2)]
                for jc in range(njc):
                    rel = jc - 4 * ic
                    ls = 0 if rel < 0 else rel * 128
                    e_t = [p2e.tile([128, 512], BF16, tag="e_t", name=f"e_{p}_{ic}_{jc}_{i}") for i in range(2)]
                    for hh in range(2):
                        pof = hh * 64
                        s_ps = sps.tile([128, 512], F32, tag="s_ps", name=f"s_{p}_{ic}_{jc}_{hh}")
                        nc.tensor.matmul(
                            s_ps[:],
                            qkT[pof:pof + 64, kof + jc * 128:kof + (jc + 1) * 128],
                            qkT[pof:pof + 64, qof + ic * 512:qof + (ic + 1) * 512],
                            start=True, stop=True,
                        )
                        if ls > 0:
                            nc.gpsimd.memset(e_t[hh][:, 0:ls], 0.0)
                        nc.scalar.activation(e_t[hh][:, ls:512], s_ps[:, ls:512], EXP)
                        if rel >= 0:
                            tri_slice = slice(rel * 128, (rel + 1) * 128)
                            nc.vector.tensor_mul(e_t[hh][:, tri_slice], e_t[hh][:, tri_slice], tri_sb[:])
                    for hh in range(2):
                        nc.tensor.matmul(
                            av[hh][:],
                            v_sb[:, jc * 260 + (2 * p + hh) * 65:jc * 260 + (2 * p + hh) * 65 + 65],
                            e_t[hh][:],
                            start=(jc == 0), stop=(jc == njc - 1),
                            skip_group_check=True,
                        )
                    if fillers and (jc % max(1, njc // len(fillers)) == 0 or jc == njc - 1):
                        while fillers and len(fillers) > (njc - 1 - jc):
                            fillers.pop(0)()
                for hh in range(2):
                    head = 2 * p + hh
                    den = p2r.tile([1, 512], F32, tag="den", name=f"den_{p}_{ic}_{hh}")
                    nc.vector.tensor_copy(den[:], av[hh][64:65, :])
                    rec = p2r.tile([1, 512], F32, tag="rec", name=f"rec_{p}_{ic}_{hh}")
                    rsc = p2r.tile([1, 512], F32, tag="rsc", name=f"rsc_{p}_{ic}_{hh}")
                    nc.vector.reciprocal_approx_accurate(rec[:], den[:], rsc[:])
                    bc_sb = p2bc.tile([64, 512], F32, tag="bc_sb", name=f"bc_{p}_{ic}_{hh}")
                    nc.gpsimd.partition_broadcast(bc_sb[:], rec[:], channels=64)
                    cof = (head // 2) * T
                    pof = (head % 2) * 64
                    nc.vector.tensor_mul(
                        attnT[pof:pof + 64, cof + ic * 512:cof + (ic + 1) * 512],
                        av[hh][0:64, :], bc_sb[:],
                    )

            def out_proj_chunk(tcc):
                tsl = slice(tcc * 128, (tcc + 1) * 128)
                for oc in range(2):
                    po = pj.tile([128, 512], F32, tag="pj", name=f"po_{tcc}_{oc}")
                    for cc in range(2):
                        nc.tensor.matmul(
                            po[:],
                            attnT[:, cc * T + tcc * 128:cc * T + (tcc + 1) * 128],
                            wout_sb[:, cc * D + oc * 512:cc * D + (oc + 1) * 512],
                            start=(cc == 0), stop=(cc == 1),
                        )
                    osl = slice(oc * 512, (oc + 1) * 512)
                    po_sb = p1t.tile([128, 512], F32, tag="tmp_c", name=f"po_sb_{tcc}_{oc}")
                    if oc == 0:
                        nc.vector.tensor_copy(po_sb[:], po[:])
                    else:
                        nc.scalar.copy(po_sb[:], po[:])
                    nc.sync.dma_start(OUT[tsl, osl], po_sb[:])

            # ---------------- schedule: n-major waves -------------------
            # wave n: project all qk m-chunks + v chunks for t-block n, run
            # both pairs' attention for query block n, and the out
            # projection for t-chunks completed in wave n-1.
            for m in range(4):
                qk_proj_chunk(m, 0)
            for tcc in range(4):
                v_proj_chunk(tcc)
            for n in range(TC512):
                fill0, fill1 = [], []
                if n < 3:
                    nx = n + 1
                    fill0 += [(lambda m=m: qk_proj_chunk(m, nx)) for m in range(4)]
                    fill0 += [(lambda t=t: v_proj_chunk(t)) for t in range(4 * nx, 4 * nx + 4)]
                if n >= 1:
                    fill1 += [(lambda t=t: out_proj_chunk(t)) for t in range(4 * (n - 1), 4 * (n - 1) + 4)]
                half = len(fill0) // 2
                attn_ic(0, n, fill0[:half] + fill1[:2])
                attn_ic(1, n, fill0[half:] + fill1[2:])
            for tcc in range(12, 16):
                out_proj_chunk(tcc)

            if debug:
                nc.sync.dma_start(DBG_QKT[:], qkT[:].bitcast(F32))
                nc.sync.dma_start(DBG_V[:], v_sb[:])
                nc.sync.dma_start(DBG_ATT[:], attnT[:].bitcast(F32))

    nc.compile()
    return nc


_DEINT = list(range(0, DK, 2)) + list(range(1, DK, 2))


def _rope_tables():
    j = np.arange(DK // 2, dtype=np.float64)
    inv_freq = THETA ** (-2.0 * j / DK)
    t = np.arange(T, dtype=np.float64)
    ang = t[None, :] * inv_freq[:, None]          # [32, T]
    ang = np.tile(ang, (4, 1))                    # [128, T]
    return np.cos(ang).astype(np.float32), np.sin(ang).astype(np.float32)


def _psw():
    M = np.zeros((128, 128), dtype=np.float32)
    for p in range(128):
        pm = p % 64
        if pm < 32:
            M[p, p + 32] = -1.0
        else:
            M[p, p - 32] = 1.0
    return np.ascontiguousarray(M.T)


def shard_inputs(x, Wqkv, bqkv, Wout, bout):
    x = np.asarray(x, dtype=np.float32)
    Wqkv = np.asarray(Wqkv, dtype=np.float32)
    bqkv = np.asarray(bqkv, dtype=np.float32)
    Wout = np.asarray(Wout, dtype=np.float32)

    cos_t, sin_t = _rope_tables()
    cq = np.ascontiguousarray(cos_t / 8.0)
    sq = np.ascontiguousarray(sin_t / 8.0)
    psw = _psw()
    tri = np.triu(np.ones((128, 128), dtype=np.float32)).astype(ml_dtypes.bfloat16)
    ones64 = np.ones((1, 64), dtype=np.float32)

    Wfull = np.concatenate([Wqkv, bqkv[:, None]], axis=1)  # [3072, 1025]

    xt = {}
    for b in range(B):
        xt[b] = np.ascontiguousarray(
            np.concatenate([x[b].T, np.ones((1, T), np.float32)], axis=0)
        )

    in_maps = []
    for c in range(NCORES):
        b = c // 4
        heads = [4 * (c % 4) + i for i in range(HEADS_PER_CORE)]
        # chunk order: [Qp0 | Kp0 | Qp1 | Kp1], each 128 rows (2 heads x 64)
        qk_rows = []
        for p in range(2):
            qrows, krows = [], []
            for h in (2 * p, 2 * p + 1):
                H = heads[h]
                qrows += [H * 192 + j for j in _DEINT]
                krows += [H * 192 + 64 + j for j in _DEINT]
            qk_rows += qrows + krows
        v_rows = []
        for h in range(4):
            H = heads[h]
            v_rows += [H * 192 + 128 + j for j in range(DK)]
        vch_out = []
        for h in range(4):
            H = heads[h]
            vch_out += [H * 64 + j for j in range(DK)]

        in_maps.append({
            "XT": xt[b],
            "WQK": np.ascontiguousarray(Wfull[qk_rows].T),
            "WV": np.ascontiguousarray(Wfull[v_rows].T),
            "WOUT": np.ascontiguousarray(Wout[:, vch_out].T),
            "PSW": psw,
            "CQ": cq,
            "SQ": sq,
            "TRI": tri,
            "ONES64": ones64,
        })
    return in_maps


_CACHED = {}


def _get_program(debug=False):
    key = bool(debug)
    if key not in _CACHED:
        _CACHED[key] = build_program(debug=debug)
    return _CACHED[key]


def run_cores(inputs, debug=False, trace=False, tmpdir=None):
    nc = _get_program(debug=debug)
    in_maps = shard_inputs(**inputs)
    res = run_bass_kernel_spmd(
        nc, in_maps, core_ids=list(range(NCORES)), trace=trace, tmpdir=tmpdir,
    )
    return res


def combine(results, bout):
    bout = np.asarray(bout, dtype=np.float32)
    out = np.empty((B, T, D), dtype=np.float32)
    for b in range(B):
        acc = results[4 * b]["OUT"].astype(np.float32).copy()
        for c in range(4 * b + 1, 4 * b + 4):
            acc += results[c]["OUT"]
        out[b] = acc + bout[None, :]
    return out


def kernel(x, Wqkv, bqkv, Wout, bout):
    res = run_cores(dict(x=x, Wqkv=Wqkv, bqkv=bqkv, Wout=Wout, bout=bout))
    return combine(res.results, bout)


# revision 16
# speedup vs baseline: 1.4574x; 1.0404x over previous
"""Causal self-attention with rotary embeddings (B=2, T=2048, D=1024, H=16,
d_k=64) on 8 Trainium2 NeuronCores.

Sharding: core c handles batch b = c//4 and 4 heads (c%4)*4..+4 — data
parallel on B, tensor parallel on heads.  Each core computes its heads'
qkv projection, RoPE, causal attention, and a partial output projection
over its 256 attention channels; the host sums the 4 partials per batch.

Layout tricks:
  * q/k channels are de-interleaved host-side (RoPE pair -> half-split
    form) and packed 2 heads per 128-partition tile; scores matmuls are
    row-tiled K=64 pairs.
  * RoPE swap (+/- sign) is a 128x128 permutation matmul on TensorE; the
    cos/sin elementwise work runs on VectorE fused with PSUM eviction.
  * softmax skips max-subtraction (scores ~ N(0,1), bounded) and folds the
    denominator into attn@v as an extra ones-column of v; the divide is a
    per-head broadcast-reciprocal multiply at eviction.
  * all matmul inputs are float32r (TF32-rate on TensorE at full fp32
    memory layout); exp outputs / v are bf16.
"""

import sys

sys.path.insert(0, "/opt/trn_rl_repo")

import numpy as np
import ml_dtypes

import concourse.bacc as bacc
import concourse.tile as tile
from concourse import mybir
from concourse.bass_utils import run_bass_kernel_spmd

F32 = mybir.dt.float32
F32R = mybir.dt.float32r
BF16 = mybir.dt.bfloat16

B, T, D = 2, 2048, 1024
NH, DK = 16, 64
THETA = 10000.0
NCORES = 8
HEADS_PER_CORE = 4

TC512 = T // 512        # 4   i-chunks of 512
TC128 = T // 128        # 16  t/j-chunks of 128
KC = D // 128           # 8   d_model contraction chunks


def build_program(debug=False):
    nc = bacc.Bacc("TRN2", target_bir_lowering=False, debug=False)

    XT = nc.dram_tensor("XT", [D + 1, T], F32R, kind="ExternalInput").ap()
    WQK = nc.dram_tensor("WQK", [D + 1, 512], F32R, kind="ExternalInput").ap()
    WV = nc.dram_tensor("WV", [D + 1, 256], F32R, kind="ExternalInput").ap()
    WOUT = nc.dram_tensor("WOUT", [256, D], F32R, kind="ExternalInput").ap()
    PSW = nc.dram_tensor("PSW", [128, 128], F32R, kind="ExternalInput").ap()
    CQ = nc.dram_tensor("CQ", [128, T], F32, kind="ExternalInput").ap()
    SQ = nc.dram_tensor("SQ", [128, T], F32, kind="ExternalInput").ap()
    TRI = nc.dram_tensor("TRI", [128, 128], BF16, kind="ExternalInput").ap()
    ONES64 = nc.dram_tensor("ONES64", [1, 64], F32R, kind="ExternalInput").ap()
    OUT = nc.dram_tensor("OUT", [T, D], F32, kind="ExternalOutput").ap()
    if debug:
        DBG_QKT = nc.dram_tensor("DBG_QKT", [128, 4 * T], F32, kind="ExternalOutput").ap()
        DBG_V = nc.dram_tensor("DBG_V", [128, TC128 * 260], BF16, kind="ExternalOutput").ap()
        DBG_ATT = nc.dram_tensor("DBG_ATT", [128, 2 * T], F32, kind="ExternalOutput").ap()

    MUL = mybir.AluOpType.mult
    EXP = mybir.ActivationFunctionType.Exp

    with tile.TileContext(nc) as tc:
        with (
            tc.tile_pool(name="persist", bufs=1) as persist,
            tc.tile_pool(name="p1w", bufs=1) as p1w,
            tc.tile_pool(name="p1t", bufs=3) as p1t,
            tc.tile_pool(name="p2e", bufs=5) as p2e,
            tc.tile_pool(name="p2bc", bufs=2) as p2bc,
            tc.tile_pool(name="p2r", bufs=1) as p2r,
                        tc.tile_pool(name="pj", bufs=2, space="PSUM") as pj,
            tc.tile_pool(name="sps", bufs=3, space="PSUM") as sps,
            tc.tile_pool(name="avps", bufs=3, space="PSUM") as avps,
        ):
            # ---- persistent tiles --------------------------------------
            qkT = persist.tile([128, 4 * T], F32R, tag="qkT")       # Qp0 Kp0 Qp1 Kp1
            v_sb = persist.tile([128, TC128 * 260], BF16, tag="v_sb")  # [jc, head, 64+1]
            attnT = persist.tile([128, 2 * T], F32R, tag="attnT")   # c-chunks x t
            wout_sb = persist.tile([128, 2 * D], F32R, tag="wout_sb")
            tri_sb = persist.tile([128, 128], BF16, tag="tri_sb")

            x_sb = p1w.tile([128, KC * T], F32R, tag="x_sb")
            xlast = p1w.tile([1, T], F32R, tag="xlast")
            wqk_sb = p1w.tile([128, KC * 512], F32R, tag="wqk_sb")
            wqk_last = p1w.tile([1, 512], F32R, tag="wqk_last")
            wv_sb = p1w.tile([128, KC * 256], F32R, tag="wv_sb")
            wv_last = p1w.tile([1, 256], F32R, tag="wv_last")
            psw_sb = p1w.tile([128, 128], F32R, tag="psw_sb")
            cq_sb = p1w.tile([128, T], F32, tag="cq_sb")
            sq_sb = p1w.tile([128, T], F32, tag="sq_sb")

            def load_x_block(n):
                for k in range(KC):
                    nc.sync.dma_start(
                        x_sb[:, k * T + n * 512:k * T + (n + 1) * 512],
                        XT[k * 128:(k + 1) * 128, n * 512:(n + 1) * 512])

            # interleave wqk/x0 per k-chunk so the first proj matmuls can
            # start after ~2 chunks instead of the full 4 MB
            for k in range(KC):
                nc.sync.dma_start(wqk_sb[:, k * 512:(k + 1) * 512], WQK[k * 128:(k + 1) * 128, :])
                nc.sync.dma_start(
                    x_sb[:, k * T:k * T + 512], XT[k * 128:(k + 1) * 128, 0:512])
            nc.sync.dma_start(wqk_last[:], WQK[D:D + 1, :])
            nc.sync.dma_start(xlast[:], XT[D:D + 1, :])
            nc.sync.dma_start(psw_sb[:], PSW[:])
            nc.sync.dma_start(cq_sb[:], CQ[:])
            nc.sync.dma_start(sq_sb[:], SQ[:])
            nc.sync.dma_start(tri_sb[:], TRI[:])
            for k in range(KC):
                nc.sync.dma_start(wv_sb[:, k * 256:(k + 1) * 256], WV[k * 128:(k + 1) * 128, :])
            nc.sync.dma_start(wv_last[:], WV[D:D + 1, :])

            # ones columns of v_aug: one strided memset
            v4 = v_sb[:].rearrange("p (jc h e) -> p jc h e", jc=TC128, h=4)
            nc.vector.memset(v4[:, :, :, 64:65], 1.0)

            # ---------------- building blocks ---------------------------
            def qk_proj_chunk(m, n):
                """project q/k m-chunk (128 channels) for t-chunk n (512), apply rope."""
                is_q = (m % 2 == 0)
                nsl = slice(n * 512, (n + 1) * 512)
                ps = pj.tile([128, 512], F32, tag="pj", name=f"psqk_{m}_{n}")
                for k in range(KC):
                    nc.tensor.matmul(
                        ps[:],
                        wqk_sb[:, k * 512 + m * 128:k * 512 + (m + 1) * 128],
                        x_sb[:, k * T + n * 512:k * T + (n + 1) * 512],
                        start=(k == 0), stop=False,
                    )
                nc.tensor.matmul(
                    ps[:], wqk_last[:, m * 128:(m + 1) * 128], xlast[:, nsl],
                    start=False, stop=True,
                )
                tmp_s = p1t.tile([128, 512], F32R, tag="tmp_s", name=f"tmps_{m}_{n}")
                tmp_c = p1t.tile([128, 512], F32, tag="tmp_c", name=f"tmpc_{m}_{n}")
                if is_q:
                    nc.vector.tensor_mul(tmp_s[:], ps[:], sq_sb[:, nsl])
                    nc.vector.tensor_mul(tmp_c[:], ps[:], cq_sb[:, nsl])
                else:
                    nc.vector.scalar_tensor_tensor(tmp_s[:], ps[:], 8.0, sq_sb[:, nsl], MUL, MUL)
                    nc.vector.scalar_tensor_tensor(tmp_c[:], ps[:], 8.0, cq_sb[:, nsl], MUL, MUL)
                sw = pj.tile([128, 512], F32, tag="pj", name=f"sw_{m}_{n}")
                nc.tensor.matmul(sw[:], psw_sb[:], tmp_s[:], start=True, stop=True)
                nc.vector.tensor_add(qkT[:, m * T + n * 512:m * T + (n + 1) * 512], sw[:], tmp_c[:])

            def v_proj_chunk(tcc):
                tsl = slice(tcc * 128, (tcc + 1) * 128)
                psv = pj.tile([128, 256], F32, tag="pj", name=f"psv_{tcc}")
                for k in range(KC):
                    nc.tensor.matmul(
                        psv[:],
                        x_sb[:, k * T + tcc * 128:k * T + (tcc + 1) * 128],
                        wv_sb[:, k * 256:(k + 1) * 256],
                        start=(k == 0), stop=False,
                    )
                nc.tensor.matmul(psv[:], xlast[:, tsl], wv_last[:], start=False, stop=True)
                vdst = v_sb[:, tcc * 260:(tcc + 1) * 260].rearrange(
                    "p (h e) -> p h e", h=4)[:, :, 0:64]
                vsrc = psv[:].rearrange("p (h e) -> p h e", e=64)
                nc.vector.tensor_copy(vdst, vsrc)

            def attn_ic(p, ic, fillers=()):
                """attention for head-pair p, query chunk ic (512 queries).
                fillers: callables run one per jc iteration (PE density)."""
                fillers = list(fillers)
                qof = (2 * p) * T
                kof = (2 * p + 1) * T
                njc = 4 * ic + 4
                av = [avps.tile([65, 512], F32, tag="av", name=f"av_{p}_{ic}_{i}") for i in range(2)]
                for jc in range(njc):
                    rel = jc - 4 * ic
                    ls = 0 if rel < 0 else rel * 128
                    e_t = [p2e.tile([128, 512], BF16, tag="e_t", name=f"e_{p}_{ic}_{jc}_{i}") for i in range(2)]
                    for hh in range(2):
                        pof = hh * 64
                        s_ps = sps.tile([128, 512], F32, tag="s_ps", name=f"s_{p}_{ic}_{jc}_{hh}")
                        nc.tensor.matmul(
                            s_ps[:],
                            qkT[pof:pof + 64, kof + jc * 128:kof + (jc + 1) * 128],
                            qkT[pof:pof + 64, qof + ic * 512:qof + (ic + 1) * 512],
                            start=True, stop=True,
                        )
                        if ls > 0:
                            nc.gpsimd.memset(e_t[hh][:, 0:ls], 0.0)
                        nc.scalar.activation(e_t[hh][:, ls:512], s_ps[:, ls:512], EXP)
                        if rel >= 0:
                            tri_slice = slice(rel * 128, (rel + 1) * 128)
                            nc.vector.tensor_mul(e_t[hh][:, tri_slice], e_t[hh][:, tri_slice], tri_sb[:])
                    for hh in range(2):
                        nc.tensor.matmul(
                            av[hh][:],
                            v_sb[:, jc * 260 + (2 * p + hh) * 65:jc * 260 + (2 * p + hh) * 65 + 65],
                            e_t[hh][:],
                            start=(jc == 0), stop=(jc == njc - 1),
                            skip_group_check=True,
                        )
                    if fillers and (jc % max(1, njc // len(fillers)) == 0 or jc == njc - 1):
                        while fillers and len(fillers) > (njc - 1 - jc):
                            fillers.pop(0)()
                for hh in range(2):
                    head = 2 * p + hh
                    den = p2r.tile([1, 512], F32, tag="den", name=f"den_{p}_{ic}_{hh}")
                    nc.vector.tensor_copy(den[:], av[hh][64:65, :])
                    rec = p2r.tile([1, 512], F32, tag="rec", name=f"rec_{p}_{ic}_{hh}")
                    rsc = p2r.tile([1, 512], F32, tag="rsc", name=f"rsc_{p}_{ic}_{hh}")
                    nc.vector.reciprocal_approx_accurate(rec[:], den[:], rsc[:])
                    bc_sb = p2bc.tile([64, 512], F32, tag="bc_sb", name=f"bc_{p}_{ic}_{hh}")
                    nc.gpsimd.partition_broadcast(bc_sb[:], rec[:], channels=64)
                    cof = (head // 2) * T
                    pof = (head % 2) * 64
                    nc.vector.tensor_mul(
                        attnT[pof:pof + 64, cof + ic * 512:cof + (ic + 1) * 512],
                        av[hh][0:64, :], bc_sb[:],
                    )

            def out_proj_chunk(tcc):
                tsl = slice(tcc * 128, (tcc + 1) * 128)
                for oc in range(2):
                    po = pj.tile([128, 512], F32, tag="pj", name=f"po_{tcc}_{oc}")
                    for cc in range(2):
                        nc.tensor.matmul(
                            po[:],
                            attnT[:, cc * T + tcc * 128:cc * T + (tcc + 1) * 128],
                            wout_sb[:, cc * D + oc * 512:cc * D + (oc + 1) * 512],
                            start=(cc == 0), stop=(cc == 1),
                        )
                    osl = slice(oc * 512, (oc + 1) * 512)
                    po_sb = p1t.tile([128, 512], F32, tag="tmp_c", name=f"po_sb_{tcc}_{oc}")
                    if oc == 0:
                        nc.vector.tensor_copy(po_sb[:], po[:])
                    else:
                        nc.scalar.copy(po_sb[:], po[:])
                    nc.sync.dma_start(OUT[tsl, osl], po_sb[:])

            # ---------------- schedule: n-major waves -------------------
            # wave n: project all qk m-chunks + v chunks for t-block n, run
            # both pairs' attention for query block n, and the out
            # projection for t-chunks completed in wave n-1.
            load_x_block(1)
            for m in range(4):
                qk_proj_chunk(m, 0)
            for tcc in range(4):
                v_proj_chunk(tcc)
            for n in range(TC512):
                fill0, fill1 = [], []
                if n < 3:
                    nx = n + 1
                    if nx + 1 < TC512:
                        fill0 += [lambda b=nx + 1: load_x_block(b)]
                    fill0 += [(lambda m=m: qk_proj_chunk(m, nx)) for m in range(4)]
                    fill0 += [(lambda t=t: v_proj_chunk(t)) for t in range(4 * nx, 4 * nx + 4)]
                if n == 0:
                    def load_wout():
                        for cc in range(2):
                            nc.sync.dma_start(wout_sb[:, cc * D:(cc + 1) * D], WOUT[cc * 128:(cc + 1) * 128, :])
                    fill0 += [load_wout]
                if n >= 1:
                    fill1 += [(lambda t=t: out_proj_chunk(t)) for t in range(4 * (n - 1), 4 * (n - 1) + 4)]
                half = len(fill0) // 2
                attn_ic(0, n, fill0[:half] + fill1[:2])
                attn_ic(1, n, fill0[half:] + fill1[2:])
            for tcc in range(12, 16):
                out_proj_chunk(tcc)

            if debug:
                nc.sync.dma_start(DBG_QKT[:], qkT[:].bitcast(F32))
                nc.sync.dma_start(DBG_V[:], v_sb[:])
                nc.sync.dma_start(DBG_ATT[:], attnT[:].bitcast(F32))

    nc.compile()
    return nc


_DEINT = list(range(0, DK, 2)) + list(range(1, DK, 2))


def _rope_tables():
    j = np.arange(DK // 2, dtype=np.float64)
    inv_freq = THETA ** (-2.0 * j / DK)
    t = np.arange(T, dtype=np.float64)
    ang = t[None, :] * inv_freq[:, None]          # [32, T]
    ang = np.tile(ang, (4, 1))                    # [128, T]
    return np.cos(ang).astype(np.float32), np.sin(ang).astype(np.float32)


def _psw():
    M = np.zeros((128, 128), dtype=np.float32)
    for p in range(128):
        pm = p % 64
        if pm < 32:
            M[p, p + 32] = -1.0
        else:
            M[p, p - 32] = 1.0
    return np.ascontiguousarray(M.T)


def shard_inputs(x, Wqkv, bqkv, Wout, bout):
    x = np.asarray(x, dtype=np.float32)
    Wqkv = np.asarray(Wqkv, dtype=np.float32)
    bqkv = np.asarray(bqkv, dtype=np.float32)
    Wout = np.asarray(Wout, dtype=np.float32)

    cos_t, sin_t = _rope_tables()
    cq = np.ascontiguousarray(cos_t / 8.0)
    sq = np.ascontiguousarray(sin_t / 8.0)
    psw = _psw()
    tri = np.triu(np.ones((128, 128), dtype=np.float32)).astype(ml_dtypes.bfloat16)
    ones64 = np.ones((1, 64), dtype=np.float32)

    Wfull = np.concatenate([Wqkv, bqkv[:, None]], axis=1)  # [3072, 1025]

    xt = {}
    for b in range(B):
        xt[b] = np.ascontiguousarray(
            np.concatenate([x[b].T, np.ones((1, T), np.float32)], axis=0)
        )

    in_maps = []
    for c in range(NCORES):
        b = c // 4
        heads = [4 * (c % 4) + i for i in range(HEADS_PER_CORE)]
        # chunk order: [Qp0 | Kp0 | Qp1 | Kp1], each 128 rows (2 heads x 64)
        qk_rows = []
        for p in range(2):
            qrows, krows = [], []
            for h in (2 * p, 2 * p + 1):
                H = heads[h]
                qrows += [H * 192 + j for j in _DEINT]
                krows += [H * 192 + 64 + j for j in _DEINT]
            qk_rows += qrows + krows
        v_rows = []
        for h in range(4):
            H = heads[h]
            v_rows += [H * 192 + 128 + j for j in range(DK)]
        vch_out = []
        for h in range(4):
            H = heads[h]
            vch_out += [H * 64 + j for j in range(DK)]

        in_maps.append({
            "XT": xt[b],
            "WQK": np.ascontiguousarray(Wfull[qk_rows].T),
            "WV": np.ascontiguousarray(Wfull[v_rows].T),
            "WOUT": np.ascontiguousarray(Wout[:, vch_out].T),
            "PSW": psw,
            "CQ": cq,
            "SQ": sq,
            "TRI": tri,
            "ONES64": ones64,
        })
    return in_maps


_CACHED = {}


def _get_program(debug=False):
    key = bool(debug)
    if key not in _CACHED:
        _CACHED[key] = build_program(debug=debug)
    return _CACHED[key]


def run_cores(inputs, debug=False, trace=False, tmpdir=None):
    nc = _get_program(debug=debug)
    in_maps = shard_inputs(**inputs)
    res = run_bass_kernel_spmd(
        nc, in_maps, core_ids=list(range(NCORES)), trace=trace, tmpdir=tmpdir,
    )
    return res


def combine(results, bout):
    bout = np.asarray(bout, dtype=np.float32)
    out = np.empty((B, T, D), dtype=np.float32)
    for b in range(B):
        acc = results[4 * b]["OUT"].astype(np.float32).copy()
        for c in range(4 * b + 1, 4 * b + 4):
            acc += results[c]["OUT"]
        out[b] = acc + bout[None, :]
    return out


def kernel(x, Wqkv, bqkv, Wout, bout):
    res = run_cores(dict(x=x, Wqkv=Wqkv, bqkv=bqkv, Wout=Wout, bout=bout))
    return combine(res.results, bout)


# revision 18
# speedup vs baseline: 1.4697x; 1.0085x over previous
"""Causal self-attention with rotary embeddings (B=2, T=2048, D=1024, H=16,
d_k=64) on 8 Trainium2 NeuronCores.

Sharding: core c handles batch b = c//4 and 4 heads (c%4)*4..+4 — data
parallel on B, tensor parallel on heads.  Each core computes its heads'
qkv projection, RoPE, causal attention, and a partial output projection
over its 256 attention channels; the host sums the 4 partials per batch.

Layout tricks:
  * q/k channels are de-interleaved host-side (RoPE pair -> half-split
    form) and packed 2 heads per 128-partition tile; scores matmuls are
    row-tiled K=64 pairs.
  * RoPE swap (+/- sign) is a 128x128 permutation matmul on TensorE; the
    cos/sin elementwise work runs on VectorE fused with PSUM eviction.
  * softmax skips max-subtraction (scores ~ N(0,1), bounded) and folds the
    denominator into attn@v as an extra ones-column of v; the divide is a
    per-head broadcast-reciprocal multiply at eviction.
  * all matmul inputs are float32r (TF32-rate on TensorE at full fp32
    memory layout); exp outputs / v are bf16.
"""

import sys

sys.path.insert(0, "/opt/trn_rl_repo")

import numpy as np
import ml_dtypes

import concourse.bacc as bacc
import concourse.tile as tile
from concourse import mybir
from concourse.bass_utils import run_bass_kernel_spmd

F32 = mybir.dt.float32
F32R = mybir.dt.float32r
BF16 = mybir.dt.bfloat16

B, T, D = 2, 2048, 1024
NH, DK = 16, 64
THETA = 10000.0
NCORES = 8
HEADS_PER_CORE = 4

TC512 = T // 512        # 4   i-chunks of 512
TC128 = T // 128        # 16  t/j-chunks of 128
KC = D // 128           # 8   d_model contraction chunks


def build_program(debug=False):
    nc = bacc.Bacc("TRN2", target_bir_lowering=False, debug=False)

    XT = nc.dram_tensor("XT", [D + 1, T], F32R, kind="ExternalInput").ap()
    WQK = nc.dram_tensor("WQK", [D + 1, 512], F32R, kind="ExternalInput").ap()
    WV = nc.dram_tensor("WV", [D + 1, 256], F32R, kind="ExternalInput").ap()
    WOUT = nc.dram_tensor("WOUT", [256, D], F32R, kind="ExternalInput").ap()
    PSW = nc.dram_tensor("PSW", [128, 128], F32R, kind="ExternalInput").ap()
    CQ = nc.dram_tensor("CQ", [128, T], F32, kind="ExternalInput").ap()
    SQ = nc.dram_tensor("SQ", [128, T], F32, kind="ExternalInput").ap()
    TRI = nc.dram_tensor("TRI", [128, 128], BF16, kind="ExternalInput").ap()
    ONES64 = nc.dram_tensor("ONES64", [1, 64], F32R, kind="ExternalInput").ap()
    OUT = nc.dram_tensor("OUT", [T, D], F32, kind="ExternalOutput").ap()
    if debug:
        DBG_QKT = nc.dram_tensor("DBG_QKT", [128, 4 * T], F32, kind="ExternalOutput").ap()
        DBG_V = nc.dram_tensor("DBG_V", [128, TC128 * 260], BF16, kind="ExternalOutput").ap()
        DBG_ATT = nc.dram_tensor("DBG_ATT", [128, 2 * T], F32, kind="ExternalOutput").ap()

    MUL = mybir.AluOpType.mult
    EXP = mybir.ActivationFunctionType.Exp

    with tile.TileContext(nc) as tc:
        with (
            tc.tile_pool(name="persist", bufs=1) as persist,
            tc.tile_pool(name="p1w", bufs=1) as p1w,
            tc.tile_pool(name="p1t", bufs=3) as p1t,
            tc.tile_pool(name="p2e", bufs=5) as p2e,
            tc.tile_pool(name="p2bc", bufs=2) as p2bc,
            tc.tile_pool(name="p2r", bufs=1) as p2r,
                        tc.tile_pool(name="pj", bufs=2, space="PSUM") as pj,
            tc.tile_pool(name="sps", bufs=3, space="PSUM") as sps,
            tc.tile_pool(name="avps", bufs=3, space="PSUM") as avps,
        ):
            # ---- persistent tiles --------------------------------------
            qkT = persist.tile([128, 4 * T], F32R, tag="qkT")       # Qp0 Kp0 Qp1 Kp1
            v_sb = persist.tile([128, TC128 * 260], BF16, tag="v_sb")  # [jc, head, 64+1]
            attnT = persist.tile([128, 2 * T], F32R, tag="attnT")   # c-chunks x t
            wout_sb = persist.tile([128, 2 * D], F32R, tag="wout_sb")
            tri_sb = persist.tile([128, 128], BF16, tag="tri_sb")

            x_sb = p1w.tile([128, KC * T], F32R, tag="x_sb")
            xlast = p1w.tile([1, T], F32R, tag="xlast")
            wqk_sb = p1w.tile([128, KC * 512], F32R, tag="wqk_sb")
            wqk_last = p1w.tile([1, 512], F32R, tag="wqk_last")
            wv_sb = p1w.tile([128, KC * 256], F32R, tag="wv_sb")
            wv_last = p1w.tile([1, 256], F32R, tag="wv_last")
            psw_sb = p1w.tile([128, 128], F32R, tag="psw_sb")
            cq_sb = p1w.tile([128, T], F32, tag="cq_sb")
            sq_sb = p1w.tile([128, T], F32, tag="sq_sb")

            def load_x_block(n):
                for k in range(KC):
                    nc.gpsimd.dma_start(
                        x_sb[:, k * T + n * 512:k * T + (n + 1) * 512],
                        XT[k * 128:(k + 1) * 128, n * 512:(n + 1) * 512])

            # preamble loads: spread trigger issue across engines (the
            # sync queue issues one DMA trigger per ~0.6us — serializing
            # 20+ triggers would starve the first matmuls)
            for k in range(KC):
                nc.sync.dma_start(wqk_sb[:, k * 512:(k + 1) * 512], WQK[k * 128:(k + 1) * 128, :])
                nc.gpsimd.dma_start(
                    x_sb[:, k * T:k * T + 512], XT[k * 128:(k + 1) * 128, 0:512])
            nc.scalar.dma_start(wqk_last[:], WQK[D:D + 1, :])
            nc.scalar.dma_start(xlast[:], XT[D:D + 1, :])
            nc.scalar.dma_start(psw_sb[:], PSW[:])
            nc.scalar.dma_start(cq_sb[:], CQ[:])
            nc.scalar.dma_start(sq_sb[:], SQ[:])
            nc.scalar.dma_start(tri_sb[:], TRI[:])
            for k in range(KC):
                nc.gpsimd.dma_start(wv_sb[:, k * 256:(k + 1) * 256], WV[k * 128:(k + 1) * 128, :])
            nc.gpsimd.dma_start(wv_last[:], WV[D:D + 1, :])

            # ones columns of v_aug: one strided memset
            v4 = v_sb[:].rearrange("p (jc h e) -> p jc h e", jc=TC128, h=4)
            nc.vector.memset(v4[:, :, :, 64:65], 1.0)

            # ---------------- building blocks ---------------------------
            def qk_proj_chunk(m, n):
                """project q/k m-chunk (128 channels) for t-chunk n (512), apply rope."""
                is_q = (m % 2 == 0)
                nsl = slice(n * 512, (n + 1) * 512)
                ps = pj.tile([128, 512], F32, tag="pj", name=f"psqk_{m}_{n}")
                for k in range(KC):
                    nc.tensor.matmul(
                        ps[:],
                        wqk_sb[:, k * 512 + m * 128:k * 512 + (m + 1) * 128],
                        x_sb[:, k * T + n * 512:k * T + (n + 1) * 512],
                        start=(k == 0), stop=False,
                    )
                nc.tensor.matmul(
                    ps[:], wqk_last[:, m * 128:(m + 1) * 128], xlast[:, nsl],
                    start=False, stop=True,
                )
                tmp_s = p1t.tile([128, 512], F32R, tag="tmp_s", name=f"tmps_{m}_{n}")
                tmp_c = p1t.tile([128, 512], F32, tag="tmp_c", name=f"tmpc_{m}_{n}")
                if is_q:
                    nc.vector.tensor_mul(tmp_s[:], ps[:], sq_sb[:, nsl])
                    nc.vector.tensor_mul(tmp_c[:], ps[:], cq_sb[:, nsl])
                else:
                    nc.vector.scalar_tensor_tensor(tmp_s[:], ps[:], 8.0, sq_sb[:, nsl], MUL, MUL)
                    nc.vector.scalar_tensor_tensor(tmp_c[:], ps[:], 8.0, cq_sb[:, nsl], MUL, MUL)
                sw = pj.tile([128, 512], F32, tag="pj", name=f"sw_{m}_{n}")
                nc.tensor.matmul(sw[:], psw_sb[:], tmp_s[:], start=True, stop=True)
                nc.vector.tensor_add(qkT[:, m * T + n * 512:m * T + (n + 1) * 512], sw[:], tmp_c[:])

            def v_proj_chunk(tcc):
                tsl = slice(tcc * 128, (tcc + 1) * 128)
                psv = pj.tile([128, 256], F32, tag="pj", name=f"psv_{tcc}")
                for k in range(KC):
                    nc.tensor.matmul(
                        psv[:],
                        x_sb[:, k * T + tcc * 128:k * T + (tcc + 1) * 128],
                        wv_sb[:, k * 256:(k + 1) * 256],
                        start=(k == 0), stop=False,
                    )
                nc.tensor.matmul(psv[:], xlast[:, tsl], wv_last[:], start=False, stop=True)
                vdst = v_sb[:, tcc * 260:(tcc + 1) * 260].rearrange(
                    "p (h e) -> p h e", h=4)[:, :, 0:64]
                vsrc = psv[:].rearrange("p (h e) -> p h e", e=64)
                nc.vector.tensor_copy(vdst, vsrc)

            def attn_ic(p, ic, fillers=()):
                """attention for head-pair p, query chunk ic (512 queries).
                fillers: callables run one per jc iteration (PE density)."""
                fillers = list(fillers)
                qof = (2 * p) * T
                kof = (2 * p + 1) * T
                njc = 4 * ic + 4
                av = [avps.tile([65, 512], F32, tag="av", name=f"av_{p}_{ic}_{i}") for i in range(2)]
                for jc in range(njc):
                    rel = jc - 4 * ic
                    ls = 0 if rel < 0 else rel * 128
                    e_t = [p2e.tile([128, 512], BF16, tag="e_t", name=f"e_{p}_{ic}_{jc}_{i}") for i in range(2)]
                    for hh in range(2):
                        pof = hh * 64
                        s_ps = sps.tile([128, 512], F32, tag="s_ps", name=f"s_{p}_{ic}_{jc}_{hh}")
                        nc.tensor.matmul(
                            s_ps[:],
                            qkT[pof:pof + 64, kof + jc * 128:kof + (jc + 1) * 128],
                            qkT[pof:pof + 64, qof + ic * 512:qof + (ic + 1) * 512],
                            start=True, stop=True,
                        )
                        if ls > 0:
                            nc.gpsimd.memset(e_t[hh][:, 0:ls], 0.0)
                        nc.scalar.activation(e_t[hh][:, ls:512], s_ps[:, ls:512], EXP)
                        if rel >= 0:
                            tri_slice = slice(rel * 128, (rel + 1) * 128)
                            nc.vector.tensor_mul(e_t[hh][:, tri_slice], e_t[hh][:, tri_slice], tri_sb[:])
                    for hh in range(2):
                        nc.tensor.matmul(
                            av[hh][:],
                            v_sb[:, jc * 260 + (2 * p + hh) * 65:jc * 260 + (2 * p + hh) * 65 + 65],
                            e_t[hh][:],
                            start=(jc == 0), stop=(jc == njc - 1),
                            skip_group_check=True,
                        )
                    if fillers and (jc % max(1, njc // len(fillers)) == 0 or jc == njc - 1):
                        while fillers and len(fillers) > (njc - 1 - jc):
                            fillers.pop(0)()
                for hh in range(2):
                    head = 2 * p + hh
                    den = p2r.tile([1, 512], F32, tag="den", name=f"den_{p}_{ic}_{hh}")
                    nc.vector.tensor_copy(den[:], av[hh][64:65, :])
                    rec = p2r.tile([1, 512], F32, tag="rec", name=f"rec_{p}_{ic}_{hh}")
                    rsc = p2r.tile([1, 512], F32, tag="rsc", name=f"rsc_{p}_{ic}_{hh}")
                    nc.vector.reciprocal_approx_accurate(rec[:], den[:], rsc[:])
                    bc_sb = p2bc.tile([64, 512], F32, tag="bc_sb", name=f"bc_{p}_{ic}_{hh}")
                    nc.gpsimd.partition_broadcast(bc_sb[:], rec[:], channels=64)
                    cof = (head // 2) * T
                    pof = (head % 2) * 64
                    nc.vector.tensor_mul(
                        attnT[pof:pof + 64, cof + ic * 512:cof + (ic + 1) * 512],
                        av[hh][0:64, :], bc_sb[:],
                    )

            def out_proj_chunk(tcc):
                tsl = slice(tcc * 128, (tcc + 1) * 128)
                for oc in range(2):
                    po = pj.tile([128, 512], F32, tag="pj", name=f"po_{tcc}_{oc}")
                    for cc in range(2):
                        nc.tensor.matmul(
                            po[:],
                            attnT[:, cc * T + tcc * 128:cc * T + (tcc + 1) * 128],
                            wout_sb[:, cc * D + oc * 512:cc * D + (oc + 1) * 512],
                            start=(cc == 0), stop=(cc == 1),
                        )
                    osl = slice(oc * 512, (oc + 1) * 512)
                    po_sb = p1t.tile([128, 512], F32, tag="tmp_c", name=f"po_sb_{tcc}_{oc}")
                    if oc == 0:
                        nc.vector.tensor_copy(po_sb[:], po[:])
                    else:
                        nc.scalar.copy(po_sb[:], po[:])
                    nc.sync.dma_start(OUT[tsl, osl], po_sb[:])

            # ---------------- schedule: n-major waves -------------------
            # wave n: project all qk m-chunks + v chunks for t-block n, run
            # both pairs' attention for query block n, and the out
            # projection for t-chunks completed in wave n-1.
            load_x_block(1)
            for m in range(4):
                qk_proj_chunk(m, 0)
            for tcc in range(4):
                v_proj_chunk(tcc)
            for n in range(TC512):
                fill0, fill1 = [], []
                if n < 3:
                    nx = n + 1
                    if nx + 1 < TC512:
                        fill0 += [lambda b=nx + 1: load_x_block(b)]
                    fill0 += [(lambda m=m: qk_proj_chunk(m, nx)) for m in range(4)]
                    fill0 += [(lambda t=t: v_proj_chunk(t)) for t in range(4 * nx, 4 * nx + 4)]
                if n == 0:
                    def load_wout():
                        for cc in range(2):
                            nc.sync.dma_start(wout_sb[:, cc * D:(cc + 1) * D], WOUT[cc * 128:(cc + 1) * 128, :])
                    fill0 += [load_wout]
                if n >= 1:
                    fill1 += [(lambda t=t: out_proj_chunk(t)) for t in range(4 * (n - 1), 4 * (n - 1) + 4)]
                half = len(fill0) // 2
                attn_ic(0, n, fill0[:half] + fill1[:2])
                attn_ic(1, n, fill0[half:] + fill1[2:])
            for tcc in range(12, 16):
                out_proj_chunk(tcc)

            if debug:
                nc.sync.dma_start(DBG_QKT[:], qkT[:].bitcast(F32))
                nc.sync.dma_start(DBG_V[:], v_sb[:])
                nc.sync.dma_start(DBG_ATT[:], attnT[:].bitcast(F32))

    nc.compile()
    return nc


_DEINT = list(range(0, DK, 2)) + list(range(1, DK, 2))


def _rope_tables():
    j = np.arange(DK // 2, dtype=np.float64)
    inv_freq = THETA ** (-2.0 * j / DK)
    t = np.arange(T, dtype=np.float64)
    ang = t[None, :] * inv_freq[:, None]          # [32, T]
    ang = np.tile(ang, (4, 1))                    # [128, T]
    return np.cos(ang).astype(np.float32), np.sin(ang).astype(np.float32)


def _psw():
    M = np.zeros((128, 128), dtype=np.float32)
    for p in range(128):
        pm = p % 64
        if pm < 32:
            M[p, p + 32] = -1.0
        else:
            M[p, p - 32] = 1.0
    return np.ascontiguousarray(M.T)


def shard_inputs(x, Wqkv, bqkv, Wout, bout):
    x = np.asarray(x, dtype=np.float32)
    Wqkv = np.asarray(Wqkv, dtype=np.float32)
    bqkv = np.asarray(bqkv, dtype=np.float32)
    Wout = np.asarray(Wout, dtype=np.float32)

    cos_t, sin_t = _rope_tables()
    cq = np.ascontiguousarray(cos_t / 8.0)
    sq = np.ascontiguousarray(sin_t / 8.0)
    psw = _psw()
    tri = np.triu(np.ones((128, 128), dtype=np.float32)).astype(ml_dtypes.bfloat16)
    ones64 = np.ones((1, 64), dtype=np.float32)

    Wfull = np.concatenate([Wqkv, bqkv[:, None]], axis=1)  # [3072, 1025]

    xt = {}
    for b in range(B):
        xt[b] = np.ascontiguousarray(
            np.concatenate([x[b].T, np.ones((1, T), np.float32)], axis=0)
        )

    in_maps = []
    for c in range(NCORES):
        b = c // 4
        heads = [4 * (c % 4) + i for i in range(HEADS_PER_CORE)]
        # chunk order: [Qp0 | Kp0 | Qp1 | Kp1], each 128 rows (2 heads x 64)
        qk_rows = []
        for p in range(2):
            qrows, krows = [], []
            for h in (2 * p, 2 * p + 1):
                H = heads[h]
                qrows += [H * 192 + j for j in _DEINT]
                krows += [H * 192 + 64 + j for j in _DEINT]
            qk_rows += qrows + krows
        v_rows = []
        for h in range(4):
            H = heads[h]
            v_rows += [H * 192 + 128 + j for j in range(DK)]
        vch_out = []
        for h in range(4):
            H = heads[h]
            vch_out += [H * 64 + j for j in range(DK)]

        in_maps.append({
            "XT": xt[b],
            "WQK": np.ascontiguousarray(Wfull[qk_rows].T),
            "WV": np.ascontiguousarray(Wfull[v_rows].T),
            "WOUT": np.ascontiguousarray(Wout[:, vch_out].T),
            "PSW": psw,
            "CQ": cq,
            "SQ": sq,
            "TRI": tri,
            "ONES64": ones64,
        })
    return in_maps


_CACHED = {}


def _get_program(debug=False):
    key = bool(debug)
    if key not in _CACHED:
        _CACHED[key] = build_program(debug=debug)
    return _CACHED[key]


def run_cores(inputs, debug=False, trace=False, tmpdir=None):
    nc = _get_program(debug=debug)
    in_maps = shard_inputs(**inputs)
    res = run_bass_kernel_spmd(
        nc, in_maps, core_ids=list(range(NCORES)), trace=trace, tmpdir=tmpdir,
    )
    return res


def combine(results, bout):
    bout = np.asarray(bout, dtype=np.float32)
    out = np.empty((B, T, D), dtype=np.float32)
    for b in range(B):
        acc = results[4 * b]["OUT"].astype(np.float32).copy()
        for c in range(4 * b + 1, 4 * b + 4):
            acc += results[c]["OUT"]
        out[b] = acc + bout[None, :]
    return out


def kernel(x, Wqkv, bqkv, Wout, bout):
    res = run_cores(dict(x=x, Wqkv=Wqkv, bqkv=bqkv, Wout=Wout, bout=bout))
    return combine(res.results, bout)
